# revision 1
# baseline (speedup 1.0000x reference)
"""Trainium2 Bass kernel: dense-masked sliding-window attention.

nn.Module semantics (see harness reference):
    B,S,E,H,W = 1, 4096, 1024, 16, 256; D = 64
    qkv = x @ w_qkv -> q,k,v  [B,S,H,D]
    scores = q k^T / sqrt(D), masked to the sliding causal window
             (key j allowed for query i iff i-W < j <= i)
    out = softmax(scores) v  -> [B,S,E] @ w_out

Sharding: sequence-parallel over 8 NeuronCores. Core c owns queries
[512c, 512c+512) and receives a 256-row key/value halo on the left; no
collectives are needed (host concatenates the per-core output rows).

Per-core kernel layout choices:
  - x is shipped pre-transposed ([E, 768] feature-major) so every matmul
    contracts over the partition dim without on-device transposes.
  - q^T, k^T are produced transposed ([dims, seq]); v natural
    ([seq, dims]), stored interleaved with a ones column per head so the
    attention row-sums (softmax denominators) fall out of the attention
    value accumulation as an extra output row.
  - scores are built transposed ([t, s]) in key-chunk-major band groups;
    softmax skips the max-subtraction (scores are O(1) here: exp cannot
    overflow) so exp is one activation op per head; the window/causal
    mask is applied multiplicatively on exp(scores) with mask data taken
    from the allowed_mask input (any mask inside the band is honored).
  - all matmuls run in fp16 (full PE rate at any moving width; fp32r
    drops to 1/4 rate below N=256). End-to-end error vs the fp32
    reference is ~4e-4 scale-relative.
"""

import numpy as np
from contextlib import ExitStack

import concourse.bass as bass
import concourse.tile as tile
from concourse import bacc, mybir
from concourse.bass_utils import run_bass_kernel_spmd

F32 = mybir.dt.float32
F32R = mybir.dt.float32r
F16 = mybir.dt.float16

B, S, E, H, W = 1, 4096, 1024, 16, 256
D = E // H  # 64
SCALE = D ** -0.5
N_CORES = 8
SQ = S // N_CORES          # 512 queries per core
HALO = W                   # 256 halo keys
SK = SQ + HALO             # 768 key rows per core
KC = E // 128              # 8 contraction chunks
QB = SQ // 128             # 4 query blocks per core
TC = SK // 128             # 6 key chunks per core
VW = H * (D + 1)           # 1040: v row width with ones columns

# band groups, key-chunk major: for key chunk T the valid query blocks
# are qb in [T-2, T] (clipped); groups are contiguous in both the scores
# free dim and the query dim.  Group order is chosen so no group crosses
# a 512-column PSUM bank boundary.
_GORDER = [2, 0, 3, 5, 1, 4]
GRP = []  # (T, qb0, nqb, col0)
_c = 0
for _T in _GORDER:
    _q0 = max(0, _T - 2)
    _qe = min(QB - 1, _T)
    GRP.append((_T, _q0, _qe - _q0 + 1, _c))
    _c += (_qe - _q0 + 1) * 128
NSLICE_COLS = _c  # 1536

# engine-assignment knobs (tuned via TimelineSim; Pool tensor_tensor
# fails walrus lowering, so both stay off Pool)
MASK_ON_POOL = False
NORM_ON_POOL = False


def _bank_split(col0, width):
    """Split a [col0, col0+width) span at 512-col PSUM bank boundaries."""
    out = []
    c = col0
    while c < col0 + width:
        e = min(col0 + width, (c // 512 + 1) * 512)
        out.append((c, e - c))
        c = e
    return out


def _emit_body(ctx: ExitStack, tc_: "tile.TileContext", xT_d, wq_d, wk_d, wv_d,
               wout_d, pmask_d, out_d):
    nc = tc_.nc
    P = 128

    xt_pool = ctx.enter_context(tc_.tile_pool(name="xt", bufs=KC))
    w_pool = ctx.enter_context(tc_.tile_pool(name="w", bufs=10))
    qt_pool = ctx.enter_context(tc_.tile_pool(name="qt", bufs=KC))
    kt_pool = ctx.enter_context(tc_.tile_pool(name="kt", bufs=KC))
    v_pool = ctx.enter_context(tc_.tile_pool(name="v", bufs=TC))
    pm_pool = ctx.enter_context(tc_.tile_pool(name="pm", bufs=1))
    et_pool = ctx.enter_context(tc_.tile_pool(name="et", bufs=3))
    at_pool = ctx.enter_context(tc_.tile_pool(name="at", bufs=KC))
    os_pool = ctx.enter_context(tc_.tile_pool(name="os", bufs=2))
    nrm_pool = ctx.enter_context(tc_.tile_pool(name="nrm", bufs=4))
    ps_big = ctx.enter_context(tc_.tile_pool(name="psb", bufs=2, space="PSUM"))
    ps_ot = ctx.enter_context(tc_.tile_pool(name="pso", bufs=2, space="PSUM"))

    # ---- loads (wq/xt first: they gate the first matmuls) ----------------
    wq, xt = [], []
    for k in range(KC):
        t = w_pool.tile([P, 1024], F16, tag="w")
        nc.sync.dma_start(t[:], wq_d.ap()[k * P:(k + 1) * P, :])
        wq.append(t)
        t = xt_pool.tile([P, SK], F16, tag="xt")
        nc.sync.dma_start(t[:], xT_d.ap()[k * P:(k + 1) * P, :])
        xt.append(t)

    # ---- q^T [E, SQ]: stationary = w_q chunk columns, moving = x^T -------
    qt = []
    for n in range(KC):
        if n % 2 == 0:
            ps_full = ps_big.tile([P, 1536], F32, tag="psb", name=f"qtp{n}")
            ps = ps_full[:, 0:SQ]
        else:
            ps = ps_ot.tile([P, SQ], F32, tag="pso", name=f"qtp{n}")
        for k in range(KC):
            nc.tensor.matmul(ps[:, 0:SQ], wq[k][:, n * P:(n + 1) * P],
                             xt[k][:, HALO:SK], start=(k == 0), stop=(k == KC - 1))
        t = qt_pool.tile([P, SQ], F16, tag="qt")
        nc.scalar.copy(t[:], ps[:, 0:SQ])
        qt.append(t)

    # ---- k^T [E, SK] -----------------------------------------------------
    wk = []
    for k in range(KC):
        t = w_pool.tile([P, 1024], F16, tag="w")
        nc.sync.dma_start(t[:], wk_d.ap()[k * P:(k + 1) * P, :])
        wk.append(t)
    kt = []
    for n in range(KC):
        ps = ps_big.tile([P, 1536], F32, tag="psb")
        for k in range(KC):
            nc.tensor.matmul(ps[:, 0:512], wk[k][:, n * P:(n + 1) * P],
                             xt[k][:, 0:512], start=(k == 0), stop=(k == KC - 1))
            nc.tensor.matmul(ps[:, 512:768], wk[k][:, n * P:(n + 1) * P],
                             xt[k][:, 512:768], start=(k == 0), stop=(k == KC - 1))
        t = kt_pool.tile([P, SK], F16, tag="kt")
        nc.scalar.copy(t[:], ps[:, 0:SK])
        kt.append(t)

    # ---- v natural [SK, 16*(64+1)] --------------------------------------
    # w_v is shipped host-interleaved [E, 1040] with a zero column after
    # each head's 64 dims; the zero columns become the ones columns.
    wv = []
    for k in range(KC):
        t = w_pool.tile([P, 1024], F16, tag="wv")
        nc.sync.dma_start(t[:], wv_d.ap()[k * P:(k + 1) * P, :])
        wv.append(t)
    ones_f = nrm_pool.tile([P, 1], F32, tag="ones")
    nc.vector.memset(ones_f[:], 1.0)
    ones_b = nrm_pool.tile([1, 64], F16, tag="onesb")
    nc.vector.memset(ones_b[:], 1.0)
    vt = []
    for sc in range(TC):
        ps = ps_big.tile([P, 1536], F32, tag="psb")
        for k in range(KC):
            nc.tensor.matmul(ps[:, 0:512], xt[k][:, sc * P:(sc + 1) * P],
                             wv[k][:, 0:512], start=(k == 0), stop=(k == KC - 1))
            nc.tensor.matmul(ps[:, 512:1024], xt[k][:, sc * P:(sc + 1) * P],
                             wv[k][:, 512:1024], start=(k == 0), stop=(k == KC - 1))
        t = v_pool.tile([P, VW], F16, tag="v")
        nc.scalar.copy(
            t[:].rearrange("p (h c) -> p h c", h=H)[:, :, 0:D],
            ps[:, 0:1024].rearrange("p (h c) -> p h c", h=H))
        nc.vector.tensor_copy(
            t[:].rearrange("p (h c) -> p h c", h=H)[:, :, D:D + 1],
            ones_f[:, None, :].broadcast_to([P, H, 1]))
        vt.append(t)

    pm = pm_pool.tile([P, NSLICE_COLS], F16)
    nc.sync.dma_start(pm[:], pmask_d.ap()[:])

    # ---- attention (heads paired on PE row halves) -----------------------
    at = []
    for p in range(KC):  # 8 head pairs / e-chunks
        a = at_pool.tile([P, SQ], F16, tag="at")
        at.append(a)
        # QK for both heads of the pair back-to-back: the row-tiled
        # matmuls (rows 0-63 vs 64-127) run concurrently on the PE.
        sc_ps = []
        for sub in range(2):
            r0 = 64 * sub
            sp = ps_big.tile([P, 1536], F32, tag="psb")
            sc_ps.append(sp)
            for (T, q0, nq, col0) in GRP:
                nc.tensor.matmul(
                    sp[:, col0:col0 + nq * 128],
                    kt[p][r0:r0 + 64, T * P:(T + 1) * P],
                    qt[p][r0:r0 + 64, q0 * 128:(q0 + nq) * 128],
                    start=True, stop=True, tile_position=(r0, 0))
        for sub in range(2):
            h = 2 * p + sub
            r0 = 64 * sub
            et = et_pool.tile([P, 1536], F16, tag="et")
            nc.scalar.activation(et[:, 0:1536], sc_ps[sub][:, 0:1536],
                                 mybir.ActivationFunctionType.Exp)
            if MASK_ON_POOL:
                nc.gpsimd.tensor_tensor(et[:, 0:1536], et[:, 0:1536],
                                        pm[:, 0:1536], mybir.AluOpType.mult)
            else:
                nc.vector.tensor_tensor(et[:, 0:1536], et[:, 0:1536],
                                        pm[:, 0:1536], mybir.AluOpType.mult)
            # attention-value products; denominators land in row 64
            ot = ps_ot.tile([P, SQ], F32, tag="pso")
            for gi, (T, q0, nq, col0) in enumerate(GRP):
                nc.tensor.matmul(
                    ot[0:65, q0 * 128:(q0 + nq) * 128],
                    vt[T][:, h * 65:h * 65 + 65],
                    et[:, col0:col0 + nq * 128],
                    start=(gi == 0), stop=(gi == len(GRP) - 1))
            # normalize rows by the per-query denominator
            rc = nrm_pool.tile([1, SQ], F32, tag="rc")
            nc.vector.reciprocal(rc[:], ot[64:65, :])
            rb = nrm_pool.tile([64, SQ], F32, tag="rb")
            nc.gpsimd.partition_broadcast(rb[:], rc[:])
            if NORM_ON_POOL:
                nc.gpsimd.tensor_tensor(a[r0:r0 + 64, :], ot[0:64, :], rb[:],
                                        mybir.AluOpType.mult)
            else:
                nc.vector.tensor_tensor(a[r0:r0 + 64, :], ot[0:64, :], rb[:],
                                        mybir.AluOpType.mult)

    # ---- output projection ----------------------------------------------
    wo = []
    for p in range(KC):
        t = w_pool.tile([P, 1024], F16, tag="w")
        nc.sync.dma_start(t[:], wout_d.ap()[p * P:(p + 1) * P, :])
        wo.append(t)
    for sb in range(QB):
        ps = ps_big.tile([P, 1536], F32, tag="psb")
        for p in range(KC):
            nc.tensor.matmul(ps[:, 0:512], at[p][:, sb * P:(sb + 1) * P],
                             wo[p][:, 0:512], start=(p == 0), stop=(p == KC - 1))
            nc.tensor.matmul(ps[:, 512:1024], at[p][:, sb * P:(sb + 1) * P],
                             wo[p][:, 512:1024], start=(p == 0), stop=(p == KC - 1))
        ob = os_pool.tile([P, E], F16, tag="os")
        nc.scalar.copy(ob[:], ps[:, 0:E])
        nc.sync.dma_start(out_d.ap()[sb * P:(sb + 1) * P, :], ob[:])


def build(n_iters: int = 1):
    nc = bacc.Bacc("TRN2", target_bir_lowering=False, debug=False,
                   num_devices=N_CORES)
    xT_d = nc.dram_tensor("xT", [E, SK], F16, kind="ExternalInput")
    wq_d = nc.dram_tensor("wq", [E, E], F16, kind="ExternalInput")
    wk_d = nc.dram_tensor("wk", [E, E], F16, kind="ExternalInput")
    wv_d = nc.dram_tensor("wv", [E, E], F16, kind="ExternalInput")
    wout_d = nc.dram_tensor("wout", [E, E], F16, kind="ExternalInput")
    pmask_d = nc.dram_tensor("pmask", [128, NSLICE_COLS], F16,
                             kind="ExternalInput")
    out_d = nc.dram_tensor("out", [SQ, E], F16, kind="ExternalOutput")
    with tile.TileContext(nc) as tc_, ExitStack() as ctx:
        if n_iters > 1:
            with tc_.For_i(0, n_iters, 1):
                _emit_body(ctx, tc_, xT_d, wq_d, wk_d, wv_d, wout_d, pmask_d,
                           out_d)
        else:
            _emit_body(ctx, tc_, xT_d, wq_d, wk_d, wv_d, wout_d, pmask_d,
                       out_d)
    nc.compile()
    return nc


def make_in_maps(x, allowed_mask, w_qkv, w_out):
    """Shard the full inputs into per-core input maps (host marshaling)."""
    x2 = np.asarray(x, dtype=np.float32).reshape(S, E)
    wqkv = np.asarray(w_qkv, dtype=np.float32)
    wq = np.ascontiguousarray(wqkv[:, 0:E]) * np.float32(SCALE)
    wk = np.ascontiguousarray(wqkv[:, E:2 * E])
    wv = np.ascontiguousarray(wqkv[:, 2 * E:3 * E])
    wout = np.ascontiguousarray(np.asarray(w_out, dtype=np.float32))
    am = np.asarray(allowed_mask).reshape(S, S)

    xT = np.ascontiguousarray(x2.T)  # [E, S]
    in_maps = []
    for c in range(N_CORES):
        lo = c * SQ - HALO
        xTc = np.zeros((E, SK), dtype=np.float32)
        ofs = max(0, -lo)
        xTc[:, ofs:] = xT[:, lo + ofs:c * SQ + SQ]
        pmask = np.zeros((128, NSLICE_COLS), dtype=np.float32)
        for (T, q0, nq, col0) in GRP:
            t0 = lo + T * 128
            if t0 + 128 <= 0:
                continue
            tlo = max(0, -t0)
            s0 = c * SQ + q0 * 128
            blk = am[s0:s0 + nq * 128, t0 + tlo:t0 + 128]  # [s, t]
            pmask[tlo:128, col0:col0 + nq * 128] = blk.T.astype(np.float32)
        in_maps.append({
            "xT": xTc.astype(np.float16),
            "wq": wq.astype(np.float16),
            "wk": wk.astype(np.float16),
            "wv": wv.astype(np.float16),
            "wout": wout.astype(np.float16),
            "pmask": pmask.astype(np.float16),
        })
    return in_maps


_CACHED_NC = None


def kernel(x, allowed_mask, w_qkv, w_out):
    global _CACHED_NC
    if _CACHED_NC is None:
        _CACHED_NC = build()
    in_maps = make_in_maps(x, allowed_mask, w_qkv, w_out)
    res = run_bass_kernel_spmd(_CACHED_NC, in_maps, list(range(N_CORES)))
    out = np.concatenate([res.results[c]["out"].astype(np.float32)
                          for c in range(N_CORES)], axis=0)
    return out.reshape(B, S, E)



# revision 37
# speedup vs baseline: 1.1053x; 1.1053x over previous
"""Trainium2 Bass kernel: dense-masked sliding-window attention.

nn.Module semantics (see harness reference):
    B,S,E,H,W = 1, 4096, 1024, 16, 256; D = 64
    qkv = x @ w_qkv -> q,k,v  [B,S,H,D]
    scores = q k^T / sqrt(D), masked to the sliding causal window
             (key j allowed for query i iff i-W < j <= i)
    out = softmax(scores) v  -> [B,S,E] @ w_out

Sharding: sequence-parallel over 8 NeuronCores. Core c owns queries
[512c, 512c+512) and receives a 256-row key/value halo on the left; no
collectives are needed (host concatenates the per-core output rows).

Per-core kernel layout (rev2 -- attention-value swap):
  - x is shipped pre-transposed ([E, 768] feature-major) so every matmul
    contracts over the partition dim without on-device transposes.
  - q^T, k^T are produced transposed ([dims, seq]); v natural
    ([seq, dims]).  PSUM->SBUF copies are spread over ACT and DVE
    (GPSIMD cannot touch PSUM on TRN2).
  - scores are built transposed ([t, s]) in [128, 512] single-bank PSUM
    tiles; softmax skips the max-subtraction (scores are O(1): exp can't
    overflow); the window/causal mask is applied multiplicatively on
    exp(scores) with mask data taken from the allowed_mask input.
  - attention-value products run with exp(scores) STATIONARY and v
    MOVING, so the output lands [q, feat] with queries on partitions:
    the softmax denominators (via 1-column ones-matmuls against the same
    stationary weights) are per-partition values and the whole
    normalization is one reciprocal + one strided multiply per head
    pair.  A PE transpose (identity matmul) restores [feat, q] for the
    output projection.
  - all matmuls run in fp16 (full PE rate).  End-to-end error vs the
    fp32 reference is ~4e-4 scale-relative.
"""

import numpy as np
from contextlib import ExitStack

import concourse.bass as bass
import concourse.tile as tile
from concourse import bacc, mybir
from concourse.bass_utils import run_bass_kernel_spmd

F32 = mybir.dt.float32
F16 = mybir.dt.float16

B, S, E, H, W = 1, 4096, 1024, 16, 256
D = E // H  # 64
SCALE = D ** -0.5
N_CORES = 8
SQ = S // N_CORES          # 512 queries per core
HALO = W                   # 256 halo keys
SK = SQ + HALO             # 768 key rows per core
KC = E // 128              # 8 contraction chunks
QB = SQ // 128             # 4 query blocks per core
TC = SK // 128             # 6 key chunks per core

# Attention band units (T = key chunk, qb = query block), packed into
# three 512-column groups so each score tile is one PSUM bank.  Within a
# group, units sharing a T are contiguous in qb so QK needs one matmul
# per run.  col = offset in the per-head 1536-wide score/mask space.
UNITS = [
    (2, 0, 0), (2, 1, 128), (2, 2, 256), (0, 0, 384),          # block 0
    (1, 0, 512), (1, 1, 640), (4, 2, 768), (4, 3, 896),        # block 1
    (3, 1, 1024), (3, 2, 1152), (3, 3, 1280), (5, 3, 1408),    # block 2
]
# QK matmul runs per block: (T, qb0, nqb, local col0)
QK_RUNS = [
    [(2, 0, 3, 0), (0, 0, 1, 384)],
    [(1, 0, 2, 0), (4, 2, 2, 256)],
    [(3, 1, 3, 0), (5, 3, 1, 384)],
]
NSLICE_COLS = 1536


def _emit_body(ctx: ExitStack, tc_: "tile.TileContext", xT_d, wq_d, wk_d, wv_d,
               wout_d, pmask_d, ident_d, out_d):
    nc = tc_.nc
    P = 128

    xt_pool = ctx.enter_context(tc_.tile_pool(name="xt", bufs=KC))
    # 16-deep: wq+wk stay resident through the interleaved k/QK/v phase;
    # wv reuses wq's early-freed slots, wo reuses wk's (a 10-deep ring
    # deadlocks: wv DMAs would wait on wk slots whose last consumer is
    # behind the v-proj matmuls in the PE queue).
    w_pool = ctx.enter_context(tc_.tile_pool(name="w", bufs=16))
    qt_pool = ctx.enter_context(tc_.tile_pool(name="qt", bufs=KC))
    kt_pool = ctx.enter_context(tc_.tile_pool(name="kt", bufs=KC))
    v_pool = ctx.enter_context(tc_.tile_pool(name="v", bufs=TC))
    pm_pool = ctx.enter_context(tc_.tile_pool(name="pm", bufs=1))
    id_pool = ctx.enter_context(tc_.tile_pool(name="id", bufs=1))
    et_pool = ctx.enter_context(tc_.tile_pool(name="et", bufs=KC))
    at_pool = ctx.enter_context(tc_.tile_pool(name="at", bufs=1))
    atT_pool = ctx.enter_context(tc_.tile_pool(name="atT", bufs=KC))
    rc_pool = ctx.enter_context(tc_.tile_pool(name="rc", bufs=4))
    os_pool = ctx.enter_context(tc_.tile_pool(name="os", bufs=2))
    one_pool = ctx.enter_context(tc_.tile_pool(name="one", bufs=1))
    # PSUM: every working tile is one bank (score/proj/avout/transpose all
    # share the 7-deep "sc" ring); den pins the eighth bank for the whole
    # attention phase.
    sc_pool = ctx.enter_context(tc_.tile_pool(name="sc", bufs=7, space="PSUM"))
    den_pool = ctx.enter_context(tc_.tile_pool(name="den", bufs=1, space="PSUM"))

    # ---- PE warmup: junk matmuls ramp the PE p-state while DMAs land -----
    junk = one_pool.tile([P, 256], F16, tag="junk")
    nc.vector.memset(junk[:], 0.0)
    jp = sc_pool.tile([P, 256], F32, tag="sc", name="warm")
    for _ in range(14):
        nc.tensor.matmul(jp[:], junk[:, 0:128], junk[:], start=True, stop=True)

    # ---- loads (wq/xt first: they gate the first matmuls) ----------------
    wq, xt = [], []
    for k in range(KC):
        t = w_pool.tile([P, 1024], F16, tag="w")
        nc.sync.dma_start(t[:], wq_d.ap()[k * P:(k + 1) * P, :])
        wq.append(t)
        t = xt_pool.tile([P, SK], F16, tag="xt")
        nc.sync.dma_start(t[:], xT_d.ap()[k * P:(k + 1) * P, :])
        xt.append(t)

    # ---- q^T [E, SQ]: stationary = w_q chunk columns, moving = x^T -------
    qt = []
    for n in range(KC):
        ps = sc_pool.tile([P, 512], F32, tag="sc", name=f"qtp{n}")
        for k in range(KC):
            nc.tensor.matmul(ps[:], wq[k][:, n * P:(n + 1) * P],
                             xt[k][:, HALO:SK], start=(k == 0), stop=(k == KC - 1))
        t = qt_pool.tile([P, SQ], F16, tag="qt")
        nc.scalar.copy(t[:], ps[:])
        qt.append(t)

    # ---- k^T [E, SK] interleaved with attention scores -------------------
    # QK for pair p is emitted after k-proj chunk p+1 so the PE never waits
    # on the Pool copy of kt[p]; exp+mask trail on ACT/DVE.
    wk = []
    for k in range(KC):
        t = w_pool.tile([P, 1024], F16, tag="w")
        nc.sync.dma_start(t[:], wk_d.ap()[k * P:(k + 1) * P, :])
        wk.append(t)
    pm = pm_pool.tile([P, NSLICE_COLS], F16)
    nc.sync.dma_start(pm[:], pmask_d.ap()[:])
    ident = id_pool.tile([P, P], F16)
    nc.sync.dma_start(ident[:], ident_d.ap()[:])
    ones_c = one_pool.tile([P, 1], F16, tag="ones")
    nc.vector.memset(ones_c[:], 1.0)

    kt = []
    et = []

    def emit_kchunk(n):
        psa = sc_pool.tile([P, 512], F32, tag="sc", name=f"ktpa{n}")
        psb = sc_pool.tile([P, 512], F32, tag="sc", name=f"ktpb{n}")
        for k in range(KC):
            nc.tensor.matmul(psa[:], wk[k][:, n * P:(n + 1) * P],
                             xt[k][:, 0:512], start=(k == 0), stop=(k == KC - 1))
            nc.tensor.matmul(psb[:, 0:256], wk[k][:, n * P:(n + 1) * P],
                             xt[k][:, 512:768], start=(k == 0), stop=(k == KC - 1))
        t = kt_pool.tile([P, SK], F16, tag="kt")
        nc.scalar.copy(t[:, 0:512], psa[:])
        nc.vector.tensor_copy(t[:, 512:768], psb[:, 0:256])
        kt.append(t)

    def emit_qk(p):
        # et[p] holds exp(scores) for both heads of pair p: [128, 2, 1536].
        e = et_pool.tile([P, 2 * NSLICE_COLS], F16, tag="et")
        ev = e[:].rearrange("p (s c) -> p s c", s=2)
        for blk in range(3):
            sps = []
            for sub in range(2):
                r0 = 64 * sub
                sp = sc_pool.tile([P, 512], F32, tag="sc")
                sps.append(sp)
                for (T, q0, nq, c0) in QK_RUNS[blk]:
                    nc.tensor.matmul(
                        sp[:, c0:c0 + nq * 128],
                        kt[p][r0:r0 + 64, T * P:(T + 1) * P],
                        qt[p][r0:r0 + 64, q0 * 128:(q0 + nq) * 128],
                        start=True, stop=True, tile_position=(r0, 0))
            for sub in range(2):
                nc.scalar.activation(ev[:, sub, blk * 512:(blk + 1) * 512],
                                     sps[sub][:],
                                     mybir.ActivationFunctionType.Exp)
        nc.vector.tensor_tensor(
            ev[:, :, :], ev[:, :, :],
            pm[:, None, :].broadcast_to([P, 2, NSLICE_COLS]),
            mybir.AluOpType.mult)
        et.append(e)

    # ---- v natural [SK, E] (emitted interleaved below) -------------------
    wv = []
    for k in range(KC):
        t = w_pool.tile([P, 1024], F16, tag="w")
        nc.sync.dma_start(t[:], wv_d.ap()[k * P:(k + 1) * P, :])
        wv.append(t)
    vt = []

    def emit_vchunk(sc):
        psa = sc_pool.tile([P, 512], F32, tag="sc", name=f"vpa{sc}")
        psb = sc_pool.tile([P, 512], F32, tag="sc", name=f"vpb{sc}")
        for k in range(KC):
            nc.tensor.matmul(psa[:], xt[k][:, sc * P:(sc + 1) * P],
                             wv[k][:, 0:512], start=(k == 0), stop=(k == KC - 1))
            nc.tensor.matmul(psb[:], xt[k][:, sc * P:(sc + 1) * P],
                             wv[k][:, 512:1024], start=(k == 0), stop=(k == KC - 1))
        t = v_pool.tile([P, 1024], F16, tag="v")
        nc.vector.tensor_copy(t[:, 0:512], psa[:])
        nc.vector.tensor_copy(t[:, 512:1024], psb[:])
        vt.append(t)

    # PE order: k-chunks lead their pair's QK by one so the Pool copy of
    # kt[p] is never on the critical path; v-chunks slot in from pair 3 on
    # to keep the PE fed while the exp chain drains on ACT.
    emit_kchunk(0)
    for n in range(1, KC):
        emit_kchunk(n)
        emit_qk(n - 1)
        if n >= 2:
            emit_vchunk(n - 2)
    emit_qk(KC - 1)

    wo = []
    for p in range(KC):
        t = w_pool.tile([P, 1024], F16, tag="w")
        nc.sync.dma_start(t[:], wout_d.ap()[p * P:(p + 1) * P, :])
        wo.append(t)

    # ---- attention values + denominators + normalize ---------------------
    # avout: [q, (sub, qb, d)] with queries on partitions; den: one-column
    # ones-matmuls against the same stationary exp(scores).
    den = den_pool.tile([P, 64], F32)
    # at layout [q, (pair, qb, sub, d)]: the PE transpose needs each
    # (pair, qb) feature block contiguous (matmul weights APs must have a
    # single free dimension).
    at = at_pool.tile([P, 4096], F16)
    units_by_qb = [[u for u in UNITS if u[1] == qb] for qb in range(QB)]
    atT = []

    def emit_transpose(p):
        psT = sc_pool.tile([P, SQ], F16, tag="sc", name=f"tr{p}")
        for qb in range(QB):
            nc.tensor.transpose(psT[:, qb * P:(qb + 1) * P],
                                at[:, p * 512 + qb * P:p * 512 + (qb + 1) * P],
                                ident[:])
        t = atT_pool.tile([P, SQ], F16, tag="atT")
        if p % 2 == 0:
            nc.scalar.copy(t[:], psT[:])
        else:
            nc.vector.tensor_copy(t[:], psT[:])
        atT.append(t)

    for p in range(KC):
        ev = et[p][:].rearrange("p (s c) -> p s c", s=2)
        av = sc_pool.tile([P, 512], F32, tag="sc", name=f"av{p}")
        for sub in range(2):
            h = 2 * p + sub
            for qb in range(QB):
                us = units_by_qb[qb]
                for i, (T, _, c0) in enumerate(us):
                    st = ev[:, sub, c0:c0 + 128]
                    nc.tensor.matmul(
                        av[:, sub * 256 + qb * 64:sub * 256 + qb * 64 + 64],
                        st, vt[T][:, h * 64:(h + 1) * 64],
                        start=(i == 0), stop=(i == len(us) - 1))
                    nc.tensor.matmul(
                        den[:, h * 4 + qb:h * 4 + qb + 1],
                        st, ones_c[:],
                        start=(i == 0), stop=(i == len(us) - 1))
        rc = rc_pool.tile([P, 8], F32, tag="rc")
        nc.vector.reciprocal(rc[:], den[:, p * 8:(p + 1) * 8])
        nc.vector.tensor_tensor(
            at[:, p * 512:(p + 1) * 512]
                .rearrange("p (q s d) -> p q s d", q=QB, s=2),
            av[:].rearrange("p (s q d) -> p q s d", s=2, q=QB),
            rc[:].rearrange("p (s q) -> p q s", s=2)[:, :, :, None]
                 .broadcast_to([P, QB, 2, D]),
            mybir.AluOpType.mult)
    # transpose at [q, f] -> atT [f, q] via PE identity matmuls, batched
    # after the AV loop: the norm chain on DVE drains while the PE runs
    # the remaining AVs, so the transposes rarely wait.
    for p in range(KC):
        emit_transpose(p)

    # ---- output projection ----------------------------------------------
    # Blocks 0..2: one copy per half (ACT||DVE) + one DMA per half (the
    # DMAs overlap later matmuls).  Last block: 256-col column groups so
    # the copy+DMA of each group hides under the next group's matmuls and
    # the serial tail is just the final 256 columns.
    for sb in range(QB - 1):
        psa = sc_pool.tile([P, 512], F32, tag="sc", name=f"opa{sb}")
        psb = sc_pool.tile([P, 512], F32, tag="sc", name=f"opb{sb}")
        for c in range(KC):
            nc.tensor.matmul(psa[:], atT[c][:, sb * P:(sb + 1) * P],
                             wo[c][:, 0:512], start=(c == 0), stop=(c == KC - 1))
            nc.tensor.matmul(psb[:], atT[c][:, sb * P:(sb + 1) * P],
                             wo[c][:, 512:1024], start=(c == 0), stop=(c == KC - 1))
        ob = os_pool.tile([P, E], F16, tag="os")
        nc.scalar.copy(ob[:, 0:512], psa[:])
        nc.vector.tensor_copy(ob[:, 512:1024], psb[:])
        nc.sync.dma_start(out_d.ap()[sb * P:(sb + 1) * P, 0:512],
                          ob[:, 0:512])
        nc.sync.dma_start(out_d.ap()[sb * P:(sb + 1) * P, 512:1024],
                          ob[:, 512:1024])
    sb = QB - 1
    psa = sc_pool.tile([P, 512], F32, tag="sc", name="opa3")
    psb = sc_pool.tile([P, 512], F32, tag="sc", name="opb3")
    ob = os_pool.tile([P, E], F16, tag="os")
    for c in range(KC):
        nc.tensor.matmul(psa[:], atT[c][:, sb * P:(sb + 1) * P],
                         wo[c][:, 0:512], start=(c == 0), stop=(c == KC - 1))
        nc.tensor.matmul(psb[:], atT[c][:, sb * P:(sb + 1) * P],
                         wo[c][:, 512:1024], start=(c == 0), stop=(c == KC - 1))
    nc.scalar.copy(ob[:, 0:512], psa[:])
    nc.vector.tensor_copy(ob[:, 512:1024], psb[:])
    nc.sync.dma_start(out_d.ap()[sb * P:(sb + 1) * P, 0:512], ob[:, 0:512])
    nc.sync.dma_start(out_d.ap()[sb * P:(sb + 1) * P, 512:1024],
                      ob[:, 512:1024])


def build(n_iters: int = 1):
    nc = bacc.Bacc("TRN2", target_bir_lowering=False, debug=False,
                   num_devices=N_CORES)
    xT_d = nc.dram_tensor("xT", [E, SK], F16, kind="ExternalInput")
    wq_d = nc.dram_tensor("wq", [E, E], F16, kind="ExternalInput")
    wk_d = nc.dram_tensor("wk", [E, E], F16, kind="ExternalInput")
    wv_d = nc.dram_tensor("wv", [E, E], F16, kind="ExternalInput")
    wout_d = nc.dram_tensor("wout", [E, E], F16, kind="ExternalInput")
    pmask_d = nc.dram_tensor("pmask", [128, NSLICE_COLS], F16,
                             kind="ExternalInput")
    ident_d = nc.dram_tensor("ident", [128, 128], F16, kind="ExternalInput")
    out_d = nc.dram_tensor("out", [SQ, E], F16, kind="ExternalOutput")
    with tile.TileContext(nc) as tc_, ExitStack() as ctx:
        if n_iters > 1:
            with tc_.For_i(0, n_iters, 1):
                _emit_body(ctx, tc_, xT_d, wq_d, wk_d, wv_d, wout_d, pmask_d,
                           ident_d, out_d)
        else:
            _emit_body(ctx, tc_, xT_d, wq_d, wk_d, wv_d, wout_d, pmask_d,
                       ident_d, out_d)
    nc.compile()
    return nc


def make_in_maps(x, allowed_mask, w_qkv, w_out):
    """Shard the full inputs into per-core input maps (host marshaling)."""
    x2 = np.asarray(x, dtype=np.float32).reshape(S, E)
    wqkv = np.asarray(w_qkv, dtype=np.float32)
    wq = np.ascontiguousarray(wqkv[:, 0:E]) * np.float32(SCALE)
    wk = np.ascontiguousarray(wqkv[:, E:2 * E])
    wv = np.ascontiguousarray(wqkv[:, 2 * E:3 * E])
    wout = np.ascontiguousarray(np.asarray(w_out, dtype=np.float32))
    am = np.asarray(allowed_mask).reshape(S, S)
    ident = np.eye(128, dtype=np.float16)

    xT = np.ascontiguousarray(x2.T)  # [E, S]
    in_maps = []
    for c in range(N_CORES):
        lo = c * SQ - HALO
        xTc = np.zeros((E, SK), dtype=np.float32)
        ofs = max(0, -lo)
        xTc[:, ofs:] = xT[:, lo + ofs:c * SQ + SQ]
        pmask = np.zeros((128, NSLICE_COLS), dtype=np.float32)
        for (T, qb, col) in UNITS:
            t0 = lo + T * 128
            if t0 + 128 <= 0:
                continue
            tlo = max(0, -t0)
            s0 = c * SQ + qb * 128
            blk = am[s0:s0 + 128, t0 + tlo:t0 + 128]  # [s, t]
            pmask[tlo:128, col:col + 128] = blk.T.astype(np.float32)
        in_maps.append({
            "xT": xTc.astype(np.float16),
            "wq": wq.astype(np.float16),
            "wk": wk.astype(np.float16),
            "wv": wv.astype(np.float16),
            "wout": wout.astype(np.float16),
            "pmask": pmask.astype(np.float16),
            "ident": ident,
        })
    return in_maps


_CACHED_NC = None


def kernel(x, allowed_mask, w_qkv, w_out):
    global _CACHED_NC
    if _CACHED_NC is None:
        _CACHED_NC = build()
    in_maps = make_in_maps(x, allowed_mask, w_qkv, w_out)
    res = run_bass_kernel_spmd(_CACHED_NC, in_maps, list(range(N_CORES)))
    out = np.concatenate([res.results[c]["out"].astype(np.float32)
                          for c in range(N_CORES)], axis=0)
    return out.reshape(B, S, E)


# revision 41
# speedup vs baseline: 1.5910x; 1.4394x over previous
"""Trainium2 Bass kernel: dense-masked sliding-window attention.

nn.Module semantics (see harness reference):
    B,S,E,H,W = 1, 4096, 1024, 16, 256; D = 64
    qkv = x @ w_qkv -> q,k,v  [B,S,H,D]
    scores = q k^T / sqrt(D), masked to the sliding causal window
             (key j allowed for query i iff i-W < j <= i)
    out = softmax(scores) v  -> [B,S,E] @ w_out

Sharding: sequence-parallel over 8 NeuronCores. Core c owns queries
[512c, 512c+512) and receives a 256-row key/value halo on the left; no
collectives are needed (host concatenates the per-core output rows).

Per-core kernel layout (rev2 -- attention-value swap):
  - x is shipped pre-transposed ([E, 768] feature-major) so every matmul
    contracts over the partition dim without on-device transposes.
  - q^T, k^T are produced transposed ([dims, seq]); v natural
    ([seq, dims]).  PSUM->SBUF copies are spread over ACT and DVE
    (GPSIMD cannot touch PSUM on TRN2).
  - scores are built transposed ([t, s]) in [128, 512] single-bank PSUM
    tiles; softmax skips the max-subtraction (scores are O(1): exp can't
    overflow); the window/causal mask is applied multiplicatively on
    exp(scores) with mask data taken from the allowed_mask input.
  - attention-value products run with exp(scores) STATIONARY and v
    MOVING, so the output lands [q, feat] with queries on partitions:
    the softmax denominators (via 1-column ones-matmuls against the same
    stationary weights) are per-partition values and the whole
    normalization is one reciprocal + one strided multiply per head
    pair.  A PE transpose (identity matmul) restores [feat, q] for the
    output projection.
  - all matmuls run in fp16 (full PE rate).  End-to-end error vs the
    fp32 reference is ~4e-4 scale-relative.
"""

import numpy as np
from contextlib import ExitStack

import concourse.bass as bass
import concourse.tile as tile
from concourse import bacc, mybir
from concourse.bass_utils import run_bass_kernel_spmd

F32 = mybir.dt.float32
F16 = mybir.dt.float16

B, S, E, H, W = 1, 4096, 1024, 16, 256
D = E // H  # 64
SCALE = D ** -0.5
N_CORES = 8
SQ = S // N_CORES          # 512 queries per core
HALO = W                   # 256 halo keys
SK = SQ + HALO             # 768 key rows per core
KC = E // 128              # 8 contraction chunks
QB = SQ // 128             # 4 query blocks per core
TC = SK // 128             # 6 key chunks per core

# Attention band units (T = key chunk, qb = query block), packed into
# three 512-column groups so each score tile is one PSUM bank.  Within a
# group, units sharing a T are contiguous in qb so QK needs one matmul
# per run.  col = offset in the per-head 1536-wide score/mask space.
UNITS = [
    (2, 0, 0), (2, 1, 128), (2, 2, 256), (0, 0, 384),          # block 0
    (1, 0, 512), (1, 1, 640), (4, 2, 768), (4, 3, 896),        # block 1
    (3, 1, 1024), (3, 2, 1152), (3, 3, 1280), (5, 3, 1408),    # block 2
]
# QK matmul runs per block: (T, qb0, nqb, local col0)
QK_RUNS = [
    [(2, 0, 3, 0), (0, 0, 1, 384)],
    [(1, 0, 2, 0), (4, 2, 2, 256)],
    [(3, 1, 3, 0), (5, 3, 1, 384)],
]
NSLICE_COLS = 1536


def _emit_body(ctx: ExitStack, tc_: "tile.TileContext", xT_d, wq_d, wk_d, wv_d,
               wout_d, pmask_d, ident_d, out_d):
    nc = tc_.nc
    P = 128

    xt_pool = ctx.enter_context(tc_.tile_pool(name="xt", bufs=KC))
    # 16-deep: wq+wk stay resident through the interleaved k/QK/v phase;
    # wv reuses wq's early-freed slots, wo reuses wk's (a 10-deep ring
    # deadlocks: wv DMAs would wait on wk slots whose last consumer is
    # behind the v-proj matmuls in the PE queue).
    w_pool = ctx.enter_context(tc_.tile_pool(name="w", bufs=16))
    qt_pool = ctx.enter_context(tc_.tile_pool(name="qt", bufs=KC))
    kt_pool = ctx.enter_context(tc_.tile_pool(name="kt", bufs=KC))
    v_pool = ctx.enter_context(tc_.tile_pool(name="v", bufs=TC))
    pm_pool = ctx.enter_context(tc_.tile_pool(name="pm", bufs=1))
    id_pool = ctx.enter_context(tc_.tile_pool(name="id", bufs=1))
    et_pool = ctx.enter_context(tc_.tile_pool(name="et", bufs=KC))
    at_pool = ctx.enter_context(tc_.tile_pool(name="at", bufs=1))
    atT_pool = ctx.enter_context(tc_.tile_pool(name="atT", bufs=KC))
    rc_pool = ctx.enter_context(tc_.tile_pool(name="rc", bufs=4))
    os_pool = ctx.enter_context(tc_.tile_pool(name="os", bufs=2))
    one_pool = ctx.enter_context(tc_.tile_pool(name="one", bufs=1))
    # PSUM: every working tile is one bank (score/proj/avout/transpose all
    # share the 7-deep "sc" ring); den pins the eighth bank for the whole
    # attention phase.
    sc_pool = ctx.enter_context(tc_.tile_pool(name="sc", bufs=7, space="PSUM"))
    den_pool = ctx.enter_context(tc_.tile_pool(name="den", bufs=1, space="PSUM"))

    # ---- PE warmup: junk matmuls ramp the PE p-state while DMAs land -----
    junk = one_pool.tile([P, 256], F16, tag="junk")
    nc.vector.memset(junk[:], 0.0)
    jp = sc_pool.tile([P, 256], F32, tag="sc", name="warm")
    for _ in range(14):
        nc.tensor.matmul(jp[:], junk[:, 0:128], junk[:], start=True, stop=True)

    # ---- loads (wq/xt first: they gate the first matmuls) ----------------
    wq, xt = [], []
    for k in range(KC):
        t = w_pool.tile([P, 1024], F16, tag="w", bufs=8)
        nc.sync.dma_start(t[:], wq_d.ap()[k * P:(k + 1) * P, :])
        wq.append(t)
        t = xt_pool.tile([P, SK], F16, tag="xt")
        nc.sync.dma_start(t[:], xT_d.ap()[k * P:(k + 1) * P, :])
        xt.append(t)

    # ---- q^T [E, SQ]: stationary = w_q chunk columns, moving = x^T -------
    qt = []
    for n in range(KC):
        ps = sc_pool.tile([P, 512], F32, tag="sc", name=f"qtp{n}")
        for k in range(KC):
            nc.tensor.matmul(ps[:], wq[k][:, n * P:(n + 1) * P],
                             xt[k][:, HALO:SK], start=(k == 0), stop=(k == KC - 1))
        t = qt_pool.tile([P, SQ], F16, tag="qt")
        nc.scalar.copy(t[:], ps[:])
        qt.append(t)

    # ---- k^T [E, SK] interleaved with attention scores -------------------
    # QK for pair p is emitted after k-proj chunk p+1 so the PE never waits
    # on the copy of kt[p]; exp+mask trail on ACT/DVE.  wk/wv/wo are each
    # ONE chunk-interleaved [128, 8*1024] DMA: the descriptor-generation
    # cost is flat per DMA, so 3 big DMAs beat 24 chunk DMAs.
    wkt = w_pool.tile([P, KC * 1024], F16, tag="wbig", bufs=2)
    nc.sync.dma_start(wkt[:], wk_d.ap()[:])
    wk = [wkt[:, k * 1024:(k + 1) * 1024] for k in range(KC)]
    pm = pm_pool.tile([P, NSLICE_COLS], F16)
    nc.sync.dma_start(pm[:], pmask_d.ap()[:])
    ident = id_pool.tile([P, P], F16)
    nc.sync.dma_start(ident[:], ident_d.ap()[:])
    ones_c = one_pool.tile([P, 1], F16, tag="ones")
    nc.vector.memset(ones_c[:], 1.0)

    kt = []
    et = []

    def emit_kchunk(n):
        psa = sc_pool.tile([P, 512], F32, tag="sc", name=f"ktpa{n}")
        psb = sc_pool.tile([P, 512], F32, tag="sc", name=f"ktpb{n}")
        for k in range(KC):
            nc.tensor.matmul(psa[:], wk[k][:, n * P:(n + 1) * P],
                             xt[k][:, 0:512], start=(k == 0), stop=(k == KC - 1))
            nc.tensor.matmul(psb[:, 0:256], wk[k][:, n * P:(n + 1) * P],
                             xt[k][:, 512:768], start=(k == 0), stop=(k == KC - 1))
        t = kt_pool.tile([P, SK], F16, tag="kt")
        nc.scalar.copy(t[:, 0:512], psa[:])
        nc.vector.tensor_copy(t[:, 512:768], psb[:, 0:256])
        kt.append(t)

    def emit_qk(p):
        # et[p] holds exp(scores) for both heads of pair p: [128, 2, 1536].
        e = et_pool.tile([P, 2 * NSLICE_COLS], F16, tag="et")
        ev = e[:].rearrange("p (s c) -> p s c", s=2)
        for blk in range(3):
            sps = []
            for sub in range(2):
                r0 = 64 * sub
                sp = sc_pool.tile([P, 512], F32, tag="sc")
                sps.append(sp)
                for (T, q0, nq, c0) in QK_RUNS[blk]:
                    nc.tensor.matmul(
                        sp[:, c0:c0 + nq * 128],
                        kt[p][r0:r0 + 64, T * P:(T + 1) * P],
                        qt[p][r0:r0 + 64, q0 * 128:(q0 + nq) * 128],
                        start=True, stop=True, tile_position=(r0, 0))
            for sub in range(2):
                nc.scalar.activation(ev[:, sub, blk * 512:(blk + 1) * 512],
                                     sps[sub][:],
                                     mybir.ActivationFunctionType.Exp)
        nc.vector.tensor_tensor(
            ev[:, :, :], ev[:, :, :],
            pm[:, None, :].broadcast_to([P, 2, NSLICE_COLS]),
            mybir.AluOpType.mult)
        et.append(e)

    # ---- v natural [SK, E] (emitted interleaved below) -------------------
    wvt = w_pool.tile([P, KC * 1024], F16, tag="wbig", bufs=2)
    nc.sync.dma_start(wvt[:], wv_d.ap()[:])
    wv = [wvt[:, k * 1024:(k + 1) * 1024] for k in range(KC)]
    vt = []

    def emit_vchunk(sc):
        psa = sc_pool.tile([P, 512], F32, tag="sc", name=f"vpa{sc}")
        psb = sc_pool.tile([P, 512], F32, tag="sc", name=f"vpb{sc}")
        for k in range(KC):
            nc.tensor.matmul(psa[:], xt[k][:, sc * P:(sc + 1) * P],
                             wv[k][:, 0:512], start=(k == 0), stop=(k == KC - 1))
            nc.tensor.matmul(psb[:], xt[k][:, sc * P:(sc + 1) * P],
                             wv[k][:, 512:1024], start=(k == 0), stop=(k == KC - 1))
        t = v_pool.tile([P, 1024], F16, tag="v")
        nc.vector.tensor_copy(t[:, 0:512], psa[:])
        nc.vector.tensor_copy(t[:, 512:1024], psb[:])
        vt.append(t)

    # PE order: k-chunks lead their pair's QK by one so the Pool copy of
    # kt[p] is never on the critical path; v-chunks slot in from pair 3 on
    # to keep the PE fed while the exp chain drains on ACT.
    emit_kchunk(0)
    for n in range(1, KC):
        emit_kchunk(n)
        emit_qk(n - 1)
        if n >= 2:
            emit_vchunk(n - 2)
    emit_qk(KC - 1)

    wot = w_pool.tile([P, KC * 1024], F16, tag="wbig", bufs=2)
    nc.sync.dma_start(wot[:], wout_d.ap()[:])
    wo = [wot[:, k * 1024:(k + 1) * 1024] for k in range(KC)]

    # ---- attention values + denominators + normalize ---------------------
    # avout: [q, (sub, qb, d)] with queries on partitions; den: one-column
    # ones-matmuls against the same stationary exp(scores).
    den = den_pool.tile([P, 64], F32)
    # at layout [q, (pair, qb, sub, d)]: the PE transpose needs each
    # (pair, qb) feature block contiguous (matmul weights APs must have a
    # single free dimension).
    at = at_pool.tile([P, 4096], F16)
    units_by_qb = [[u for u in UNITS if u[1] == qb] for qb in range(QB)]
    atT = []

    def emit_transpose(p):
        psT = sc_pool.tile([P, SQ], F16, tag="sc", name=f"tr{p}")
        for qb in range(QB):
            nc.tensor.transpose(psT[:, qb * P:(qb + 1) * P],
                                at[:, p * 512 + qb * P:p * 512 + (qb + 1) * P],
                                ident[:])
        t = atT_pool.tile([P, SQ], F16, tag="atT")
        if p % 2 == 0:
            nc.scalar.copy(t[:], psT[:])
        else:
            nc.vector.tensor_copy(t[:], psT[:])
        atT.append(t)

    for p in range(KC):
        ev = et[p][:].rearrange("p (s c) -> p s c", s=2)
        av = sc_pool.tile([P, 512], F32, tag="sc", name=f"av{p}")
        for sub in range(2):
            h = 2 * p + sub
            for qb in range(QB):
                us = units_by_qb[qb]
                for i, (T, _, c0) in enumerate(us):
                    st = ev[:, sub, c0:c0 + 128]
                    nc.tensor.matmul(
                        av[:, sub * 256 + qb * 64:sub * 256 + qb * 64 + 64],
                        st, vt[T][:, h * 64:(h + 1) * 64],
                        start=(i == 0), stop=(i == len(us) - 1))
                    nc.tensor.matmul(
                        den[:, h * 4 + qb:h * 4 + qb + 1],
                        st, ones_c[:],
                        start=(i == 0), stop=(i == len(us) - 1))
        rc = rc_pool.tile([P, 8], F32, tag="rc")
        nc.vector.reciprocal(rc[:], den[:, p * 8:(p + 1) * 8])
        nc.vector.tensor_tensor(
            at[:, p * 512:(p + 1) * 512]
                .rearrange("p (q s d) -> p q s d", q=QB, s=2),
            av[:].rearrange("p (s q d) -> p q s d", s=2, q=QB),
            rc[:].rearrange("p (s q) -> p q s", s=2)[:, :, :, None]
                 .broadcast_to([P, QB, 2, D]),
            mybir.AluOpType.mult)
    # transpose at [q, f] -> atT [f, q] via PE identity matmuls, batched
    # after the AV loop: the norm chain on DVE drains while the PE runs
    # the remaining AVs, so the transposes rarely wait.
    for p in range(KC):
        emit_transpose(p)

    # ---- output projection ----------------------------------------------
    # Blocks 0..2: one copy per half (ACT||DVE) + one DMA per half (the
    # DMAs overlap later matmuls).  Last block: 256-col column groups so
    # the copy+DMA of each group hides under the next group's matmuls and
    # the serial tail is just the final 256 columns.
    for sb in range(QB - 1):
        psa = sc_pool.tile([P, 512], F32, tag="sc", name=f"opa{sb}")
        psb = sc_pool.tile([P, 512], F32, tag="sc", name=f"opb{sb}")
        for c in range(KC):
            nc.tensor.matmul(psa[:], atT[c][:, sb * P:(sb + 1) * P],
                             wo[c][:, 0:512], start=(c == 0), stop=(c == KC - 1))
            nc.tensor.matmul(psb[:], atT[c][:, sb * P:(sb + 1) * P],
                             wo[c][:, 512:1024], start=(c == 0), stop=(c == KC - 1))
        ob = os_pool.tile([P, E], F16, tag="os")
        nc.scalar.copy(ob[:, 0:512], psa[:])
        nc.vector.tensor_copy(ob[:, 512:1024], psb[:])
        nc.sync.dma_start(out_d.ap()[sb * P:(sb + 1) * P, 0:512],
                          ob[:, 0:512])
        nc.sync.dma_start(out_d.ap()[sb * P:(sb + 1) * P, 512:1024],
                          ob[:, 512:1024])
    sb = QB - 1
    psa = sc_pool.tile([P, 512], F32, tag="sc", name="opa3")
    psb = sc_pool.tile([P, 512], F32, tag="sc", name="opb3")
    ob = os_pool.tile([P, E], F16, tag="os")
    for c in range(KC):
        nc.tensor.matmul(psa[:], atT[c][:, sb * P:(sb + 1) * P],
                         wo[c][:, 0:512], start=(c == 0), stop=(c == KC - 1))
        nc.tensor.matmul(psb[:], atT[c][:, sb * P:(sb + 1) * P],
                         wo[c][:, 512:1024], start=(c == 0), stop=(c == KC - 1))
    nc.scalar.copy(ob[:, 0:512], psa[:])
    nc.vector.tensor_copy(ob[:, 512:1024], psb[:])
    nc.sync.dma_start(out_d.ap()[sb * P:(sb + 1) * P, 0:512], ob[:, 0:512])
    nc.sync.dma_start(out_d.ap()[sb * P:(sb + 1) * P, 512:1024],
                      ob[:, 512:1024])


def build(n_iters: int = 1):
    nc = bacc.Bacc("TRN2", target_bir_lowering=False, debug=False,
                   num_devices=N_CORES)
    xT_d = nc.dram_tensor("xT", [E, SK], F16, kind="ExternalInput")
    wq_d = nc.dram_tensor("wq", [E, E], F16, kind="ExternalInput")
    wk_d = nc.dram_tensor("wk", [128, KC * 1024], F16, kind="ExternalInput")
    wv_d = nc.dram_tensor("wv", [128, KC * 1024], F16, kind="ExternalInput")
    wout_d = nc.dram_tensor("wout", [128, KC * 1024], F16,
                            kind="ExternalInput")
    pmask_d = nc.dram_tensor("pmask", [128, NSLICE_COLS], F16,
                             kind="ExternalInput")
    ident_d = nc.dram_tensor("ident", [128, 128], F16, kind="ExternalInput")
    out_d = nc.dram_tensor("out", [SQ, E], F16, kind="ExternalOutput")
    with tile.TileContext(nc) as tc_, ExitStack() as ctx:
        if n_iters > 1:
            with tc_.For_i(0, n_iters, 1):
                _emit_body(ctx, tc_, xT_d, wq_d, wk_d, wv_d, wout_d, pmask_d,
                           ident_d, out_d)
        else:
            _emit_body(ctx, tc_, xT_d, wq_d, wk_d, wv_d, wout_d, pmask_d,
                       ident_d, out_d)
    nc.compile()
    return nc


def make_in_maps(x, allowed_mask, w_qkv, w_out):
    """Shard the full inputs into per-core input maps (host marshaling)."""
    x2 = np.asarray(x, dtype=np.float32).reshape(S, E)
    wqkv = np.asarray(w_qkv, dtype=np.float32)
    wq = np.ascontiguousarray(wqkv[:, 0:E]) * np.float32(SCALE)
    wk = np.ascontiguousarray(wqkv[:, E:2 * E])
    wv = np.ascontiguousarray(wqkv[:, 2 * E:3 * E])
    wout = np.ascontiguousarray(np.asarray(w_out, dtype=np.float32))
    am = np.asarray(allowed_mask).reshape(S, S)
    ident = np.eye(128, dtype=np.float16)

    xT = np.ascontiguousarray(x2.T)  # [E, S]
    in_maps = []
    for c in range(N_CORES):
        lo = c * SQ - HALO
        xTc = np.zeros((E, SK), dtype=np.float32)
        ofs = max(0, -lo)
        xTc[:, ofs:] = xT[:, lo + ofs:c * SQ + SQ]
        pmask = np.zeros((128, NSLICE_COLS), dtype=np.float32)
        for (T, qb, col) in UNITS:
            t0 = lo + T * 128
            if t0 + 128 <= 0:
                continue
            tlo = max(0, -t0)
            s0 = c * SQ + qb * 128
            blk = am[s0:s0 + 128, t0 + tlo:t0 + 128]  # [s, t]
            pmask[tlo:128, col:col + 128] = blk.T.astype(np.float32)
        def chunk_major(w):
            return np.ascontiguousarray(
                w.reshape(KC, 128, E).transpose(1, 0, 2).reshape(128, KC * E))
        in_maps.append({
            "xT": xTc.astype(np.float16),
            "wq": wq.astype(np.float16),
            "wk": chunk_major(wk).astype(np.float16),
            "wv": chunk_major(wv).astype(np.float16),
            "wout": chunk_major(wout).astype(np.float16),
            "pmask": pmask.astype(np.float16),
            "ident": ident,
        })
    return in_maps


_CACHED_NC = None


def kernel(x, allowed_mask, w_qkv, w_out):
    global _CACHED_NC
    if _CACHED_NC is None:
        _CACHED_NC = build()
    in_maps = make_in_maps(x, allowed_mask, w_qkv, w_out)
    res = run_bass_kernel_spmd(_CACHED_NC, in_maps, list(range(N_CORES)))
    out = np.concatenate([res.results[c]["out"].astype(np.float32)
                          for c in range(N_CORES)], axis=0)
    return out.reshape(B, S, E)


# revision 42
# speedup vs baseline: 1.7845x; 1.1217x over previous
"""Trainium2 Bass kernel: dense-masked sliding-window attention.

nn.Module semantics (see harness reference):
    B,S,E,H,W = 1, 4096, 1024, 16, 256; D = 64
    qkv = x @ w_qkv -> q,k,v  [B,S,H,D]
    scores = q k^T / sqrt(D), masked to the sliding causal window
             (key j allowed for query i iff i-W < j <= i)
    out = softmax(scores) v  -> [B,S,E] @ w_out

Sharding: sequence-parallel over 8 NeuronCores. Core c owns queries
[512c, 512c+512) and receives a 256-row key/value halo on the left; no
collectives are needed (host concatenates the per-core output rows).

Per-core kernel layout (rev2 -- attention-value swap):
  - x is shipped pre-transposed ([E, 768] feature-major) so every matmul
    contracts over the partition dim without on-device transposes.
  - q^T, k^T are produced transposed ([dims, seq]); v natural
    ([seq, dims]).  PSUM->SBUF copies are spread over ACT and DVE
    (GPSIMD cannot touch PSUM on TRN2).
  - scores are built transposed ([t, s]) in [128, 512] single-bank PSUM
    tiles; softmax skips the max-subtraction (scores are O(1): exp can't
    overflow); the window/causal mask is applied multiplicatively on
    exp(scores) with mask data taken from the allowed_mask input.
  - attention-value products run with exp(scores) STATIONARY and v
    MOVING, so the output lands [q, feat] with queries on partitions:
    the softmax denominators (via 1-column ones-matmuls against the same
    stationary weights) are per-partition values and the whole
    normalization is one reciprocal + one strided multiply per head
    pair.  A PE transpose (identity matmul) restores [feat, q] for the
    output projection.
  - all matmuls run in fp16 (full PE rate).  End-to-end error vs the
    fp32 reference is ~4e-4 scale-relative.
"""

import numpy as np
from contextlib import ExitStack

import concourse.bass as bass
import concourse.tile as tile
from concourse import bacc, mybir
from concourse.bass_utils import run_bass_kernel_spmd

F32 = mybir.dt.float32
F16 = mybir.dt.float16

B, S, E, H, W = 1, 4096, 1024, 16, 256
D = E // H  # 64
SCALE = D ** -0.5
N_CORES = 8
SQ = S // N_CORES          # 512 queries per core
HALO = W                   # 256 halo keys
SK = SQ + HALO             # 768 key rows per core
KC = E // 128              # 8 contraction chunks
QB = SQ // 128             # 4 query blocks per core
TC = SK // 128             # 6 key chunks per core

# Attention band units (T = key chunk, qb = query block), packed into
# three 512-column groups so each score tile is one PSUM bank.  Within a
# group, units sharing a T are contiguous in qb so QK needs one matmul
# per run.  col = offset in the per-head 1536-wide score/mask space.
UNITS = [
    (2, 0, 0), (2, 1, 128), (2, 2, 256), (0, 0, 384),          # block 0
    (1, 0, 512), (1, 1, 640), (4, 2, 768), (4, 3, 896),        # block 1
    (3, 1, 1024), (3, 2, 1152), (3, 3, 1280), (5, 3, 1408),    # block 2
]
# QK matmul runs per block: (T, qb0, nqb, local col0)
QK_RUNS = [
    [(2, 0, 3, 0), (0, 0, 1, 384)],
    [(1, 0, 2, 0), (4, 2, 2, 256)],
    [(3, 1, 3, 0), (5, 3, 1, 384)],
]
NSLICE_COLS = 1536


def _emit_body(ctx: ExitStack, tc_: "tile.TileContext", xT_d, wq_d, wk_d, wv_d,
               wout_d, pmask_d, ident_d, out_d):
    nc = tc_.nc
    P = 128

    xt_pool = ctx.enter_context(tc_.tile_pool(name="xt", bufs=KC))
    # 16-deep: wq+wk stay resident through the interleaved k/QK/v phase;
    # wv reuses wq's early-freed slots, wo reuses wk's (a 10-deep ring
    # deadlocks: wv DMAs would wait on wk slots whose last consumer is
    # behind the v-proj matmuls in the PE queue).
    w_pool = ctx.enter_context(tc_.tile_pool(name="w", bufs=16))
    qt_pool = ctx.enter_context(tc_.tile_pool(name="qt", bufs=KC))
    kt_pool = ctx.enter_context(tc_.tile_pool(name="kt", bufs=KC))
    v_pool = ctx.enter_context(tc_.tile_pool(name="v", bufs=TC))
    pm_pool = ctx.enter_context(tc_.tile_pool(name="pm", bufs=1))
    id_pool = ctx.enter_context(tc_.tile_pool(name="id", bufs=1))
    et_pool = ctx.enter_context(tc_.tile_pool(name="et", bufs=KC))
    at_pool = ctx.enter_context(tc_.tile_pool(name="at", bufs=1))
    atT_pool = ctx.enter_context(tc_.tile_pool(name="atT", bufs=KC))
    rc_pool = ctx.enter_context(tc_.tile_pool(name="rc", bufs=4))
    os_pool = ctx.enter_context(tc_.tile_pool(name="os", bufs=2))
    one_pool = ctx.enter_context(tc_.tile_pool(name="one", bufs=1))
    # PSUM: every working tile is one bank (score/proj/avout/transpose all
    # share the 7-deep "sc" ring); den pins the eighth bank for the whole
    # attention phase.
    sc_pool = ctx.enter_context(tc_.tile_pool(name="sc", bufs=7, space="PSUM"))
    den_pool = ctx.enter_context(tc_.tile_pool(name="den", bufs=1, space="PSUM"))

    # ---- PE warmup: junk matmuls ramp the PE p-state while DMAs land -----
    junk = one_pool.tile([P, 256], F16, tag="junk")
    nc.vector.memset(junk[:], 0.0)
    jp = sc_pool.tile([P, 256], F32, tag="sc", name="warm")
    for _ in range(14):
        nc.tensor.matmul(jp[:], junk[:, 0:128], junk[:], start=True, stop=True)

    # ---- loads (wq/xt first: they gate the first matmuls) ----------------
    wq, xt = [], []
    for k in range(KC):
        t = w_pool.tile([P, 1024], F16, tag="w", bufs=16)
        nc.sync.dma_start(t[:], wq_d.ap()[k * P:(k + 1) * P, :])
        wq.append(t)
        t = xt_pool.tile([P, SK], F16, tag="xt")
        nc.sync.dma_start(t[:], xT_d.ap()[k * P:(k + 1) * P, :])
        xt.append(t)

    # ---- q^T [E, SQ]: stationary = w_q chunk columns, moving = x^T -------
    qt = []
    for n in range(KC):
        ps = sc_pool.tile([P, 512], F32, tag="sc", name=f"qtp{n}")
        for k in range(KC):
            nc.tensor.matmul(ps[:], wq[k][:, n * P:(n + 1) * P],
                             xt[k][:, HALO:SK], start=(k == 0), stop=(k == KC - 1))
        t = qt_pool.tile([P, SQ], F16, tag="qt")
        nc.scalar.copy(t[:], ps[:])
        qt.append(t)

    # ---- k^T [E, SK] interleaved with attention scores -------------------
    # QK for pair p is emitted after k-proj chunk p+1 so the PE never waits
    # on the copy of kt[p]; exp+mask trail on ACT/DVE.
    wk = []
    for k in range(KC):
        t = w_pool.tile([P, 1024], F16, tag="w", bufs=16)
        nc.sync.dma_start(t[:], wk_d.ap()[k * P:(k + 1) * P, :])
        wk.append(t)
    pm = pm_pool.tile([P, NSLICE_COLS], F16)
    nc.sync.dma_start(pm[:], pmask_d.ap()[:])
    ident = id_pool.tile([P, P], F16)
    nc.sync.dma_start(ident[:], ident_d.ap()[:])
    ones_c = one_pool.tile([P, 1], F16, tag="ones")
    nc.vector.memset(ones_c[:], 1.0)

    kt = []
    et = []

    def emit_kchunk(n):
        psa = sc_pool.tile([P, 512], F32, tag="sc", name=f"ktpa{n}")
        psb = sc_pool.tile([P, 512], F32, tag="sc", name=f"ktpb{n}")
        for k in range(KC):
            nc.tensor.matmul(psa[:], wk[k][:, n * P:(n + 1) * P],
                             xt[k][:, 0:512], start=(k == 0), stop=(k == KC - 1))
            nc.tensor.matmul(psb[:, 0:256], wk[k][:, n * P:(n + 1) * P],
                             xt[k][:, 512:768], start=(k == 0), stop=(k == KC - 1))
        t = kt_pool.tile([P, SK], F16, tag="kt")
        nc.scalar.copy(t[:, 0:512], psa[:])
        nc.vector.tensor_copy(t[:, 512:768], psb[:, 0:256])
        kt.append(t)

    def emit_qk(p):
        # et[p] holds exp(scores) for both heads of pair p: [128, 2, 1536].
        e = et_pool.tile([P, 2 * NSLICE_COLS], F16, tag="et")
        ev = e[:].rearrange("p (s c) -> p s c", s=2)
        for blk in range(3):
            sps = []
            for sub in range(2):
                r0 = 64 * sub
                sp = sc_pool.tile([P, 512], F32, tag="sc")
                sps.append(sp)
                for (T, q0, nq, c0) in QK_RUNS[blk]:
                    nc.tensor.matmul(
                        sp[:, c0:c0 + nq * 128],
                        kt[p][r0:r0 + 64, T * P:(T + 1) * P],
                        qt[p][r0:r0 + 64, q0 * 128:(q0 + nq) * 128],
                        start=True, stop=True, tile_position=(r0, 0))
            for sub in range(2):
                nc.scalar.activation(ev[:, sub, blk * 512:(blk + 1) * 512],
                                     sps[sub][:],
                                     mybir.ActivationFunctionType.Exp)
        nc.vector.tensor_tensor(
            ev[:, :, :], ev[:, :, :],
            pm[:, None, :].broadcast_to([P, 2, NSLICE_COLS]),
            mybir.AluOpType.mult)
        et.append(e)

    # ---- v natural [SK, E] (emitted interleaved below) -------------------
    wv = []
    for k in range(KC):
        t = w_pool.tile([P, 1024], F16, tag="w", bufs=16)
        nc.sync.dma_start(t[:], wv_d.ap()[k * P:(k + 1) * P, :])
        wv.append(t)
    vt = []

    def emit_vchunk(sc):
        psa = sc_pool.tile([P, 512], F32, tag="sc", name=f"vpa{sc}")
        psb = sc_pool.tile([P, 512], F32, tag="sc", name=f"vpb{sc}")
        for k in range(KC):
            nc.tensor.matmul(psa[:], xt[k][:, sc * P:(sc + 1) * P],
                             wv[k][:, 0:512], start=(k == 0), stop=(k == KC - 1))
            nc.tensor.matmul(psb[:], xt[k][:, sc * P:(sc + 1) * P],
                             wv[k][:, 512:1024], start=(k == 0), stop=(k == KC - 1))
        t = v_pool.tile([P, 1024], F16, tag="v")
        nc.vector.tensor_copy(t[:, 0:512], psa[:])
        nc.vector.tensor_copy(t[:, 512:1024], psb[:])
        vt.append(t)

    # PE order: k-chunks lead their pair's QK by one so the Pool copy of
    # kt[p] is never on the critical path; v-chunks slot in from pair 3 on
    # to keep the PE fed while the exp chain drains on ACT.
    emit_kchunk(0)
    for n in range(1, KC):
        emit_kchunk(n)
        emit_qk(n - 1)
        if n >= 2:
            emit_vchunk(n - 2)
    emit_qk(KC - 1)

    wo = []
    for k in range(KC):
        t = w_pool.tile([P, 1024], F16, tag="w", bufs=16)
        nc.sync.dma_start(t[:], wout_d.ap()[k * P:(k + 1) * P, :])
        wo.append(t)

    # ---- attention values + denominators + normalize ---------------------
    # avout: [q, (sub, qb, d)] with queries on partitions; den: one-column
    # ones-matmuls against the same stationary exp(scores).
    den = den_pool.tile([P, 64], F32)
    # at layout [q, (pair, qb, sub, d)]: the PE transpose needs each
    # (pair, qb) feature block contiguous (matmul weights APs must have a
    # single free dimension).
    at = at_pool.tile([P, 4096], F16)
    units_by_qb = [[u for u in UNITS if u[1] == qb] for qb in range(QB)]
    atT = []

    def emit_transpose(p):
        psT = sc_pool.tile([P, SQ], F16, tag="sc", name=f"tr{p}")
        for qb in range(QB):
            nc.tensor.transpose(psT[:, qb * P:(qb + 1) * P],
                                at[:, p * 512 + qb * P:p * 512 + (qb + 1) * P],
                                ident[:])
        t = atT_pool.tile([P, SQ], F16, tag="atT")
        if p % 2 == 0:
            nc.scalar.copy(t[:], psT[:])
        else:
            nc.vector.tensor_copy(t[:], psT[:])
        atT.append(t)

    for p in range(KC):
        ev = et[p][:].rearrange("p (s c) -> p s c", s=2)
        av = sc_pool.tile([P, 512], F32, tag="sc", name=f"av{p}")
        for sub in range(2):
            h = 2 * p + sub
            for qb in range(QB):
                us = units_by_qb[qb]
                for i, (T, _, c0) in enumerate(us):
                    st = ev[:, sub, c0:c0 + 128]
                    nc.tensor.matmul(
                        av[:, sub * 256 + qb * 64:sub * 256 + qb * 64 + 64],
                        st, vt[T][:, h * 64:(h + 1) * 64],
                        start=(i == 0), stop=(i == len(us) - 1))
                    nc.tensor.matmul(
                        den[:, h * 4 + qb:h * 4 + qb + 1],
                        st, ones_c[:],
                        start=(i == 0), stop=(i == len(us) - 1))
        rc = rc_pool.tile([P, 8], F32, tag="rc")
        nc.vector.reciprocal(rc[:], den[:, p * 8:(p + 1) * 8])
        nc.vector.tensor_tensor(
            at[:, p * 512:(p + 1) * 512]
                .rearrange("p (q s d) -> p q s d", q=QB, s=2),
            av[:].rearrange("p (s q d) -> p q s d", s=2, q=QB),
            rc[:].rearrange("p (s q) -> p q s", s=2)[:, :, :, None]
                 .broadcast_to([P, QB, 2, D]),
            mybir.AluOpType.mult)
    # transpose at [q, f] -> atT [f, q] via PE identity matmuls, batched
    # after the AV loop: the norm chain on DVE drains while the PE runs
    # the remaining AVs, so the transposes rarely wait.
    for p in range(KC):
        emit_transpose(p)

    # ---- output projection ----------------------------------------------
    # Blocks 0..2: one copy per half (ACT||DVE) + one DMA per half (the
    # DMAs overlap later matmuls).  Last block: 256-col column groups so
    # the copy+DMA of each group hides under the next group's matmuls and
    # the serial tail is just the final 256 columns.
    for sb in range(QB - 1):
        psa = sc_pool.tile([P, 512], F32, tag="sc", name=f"opa{sb}")
        psb = sc_pool.tile([P, 512], F32, tag="sc", name=f"opb{sb}")
        for c in range(KC):
            nc.tensor.matmul(psa[:], atT[c][:, sb * P:(sb + 1) * P],
                             wo[c][:, 0:512], start=(c == 0), stop=(c == KC - 1))
            nc.tensor.matmul(psb[:], atT[c][:, sb * P:(sb + 1) * P],
                             wo[c][:, 512:1024], start=(c == 0), stop=(c == KC - 1))
        ob = os_pool.tile([P, E], F16, tag="os")
        nc.scalar.copy(ob[:, 0:512], psa[:])
        nc.vector.tensor_copy(ob[:, 512:1024], psb[:])
        nc.sync.dma_start(out_d.ap()[sb * P:(sb + 1) * P, 0:512],
                          ob[:, 0:512])
        nc.sync.dma_start(out_d.ap()[sb * P:(sb + 1) * P, 512:1024],
                          ob[:, 512:1024])
    sb = QB - 1
    psa = sc_pool.tile([P, 512], F32, tag="sc", name="opa3")
    psb = sc_pool.tile([P, 512], F32, tag="sc", name="opb3")
    ob = os_pool.tile([P, E], F16, tag="os")
    for c in range(KC):
        nc.tensor.matmul(psa[:], atT[c][:, sb * P:(sb + 1) * P],
                         wo[c][:, 0:512], start=(c == 0), stop=(c == KC - 1))
        nc.tensor.matmul(psb[:], atT[c][:, sb * P:(sb + 1) * P],
                         wo[c][:, 512:1024], start=(c == 0), stop=(c == KC - 1))
    nc.scalar.copy(ob[:, 0:512], psa[:])
    nc.vector.tensor_copy(ob[:, 512:1024], psb[:])
    nc.sync.dma_start(out_d.ap()[sb * P:(sb + 1) * P, 0:512], ob[:, 0:512])
    nc.sync.dma_start(out_d.ap()[sb * P:(sb + 1) * P, 512:1024],
                      ob[:, 512:1024])


def build(n_iters: int = 1):
    nc = bacc.Bacc("TRN2", target_bir_lowering=False, debug=False,
                   num_devices=N_CORES)
    xT_d = nc.dram_tensor("xT", [E, SK], F16, kind="ExternalInput")
    wq_d = nc.dram_tensor("wq", [E, E], F16, kind="ExternalInput")
    wk_d = nc.dram_tensor("wk", [E, E], F16, kind="ExternalInput")
    wv_d = nc.dram_tensor("wv", [E, E], F16, kind="ExternalInput")
    wout_d = nc.dram_tensor("wout", [E, E], F16, kind="ExternalInput")
    pmask_d = nc.dram_tensor("pmask", [128, NSLICE_COLS], F16,
                             kind="ExternalInput")
    ident_d = nc.dram_tensor("ident", [128, 128], F16, kind="ExternalInput")
    out_d = nc.dram_tensor("out", [SQ, E], F16, kind="ExternalOutput")
    with tile.TileContext(nc) as tc_, ExitStack() as ctx:
        if n_iters > 1:
            with tc_.For_i(0, n_iters, 1):
                _emit_body(ctx, tc_, xT_d, wq_d, wk_d, wv_d, wout_d, pmask_d,
                           ident_d, out_d)
        else:
            _emit_body(ctx, tc_, xT_d, wq_d, wk_d, wv_d, wout_d, pmask_d,
                       ident_d, out_d)
    nc.compile()
    return nc


def make_in_maps(x, allowed_mask, w_qkv, w_out):
    """Shard the full inputs into per-core input maps (host marshaling)."""
    x2 = np.asarray(x, dtype=np.float32).reshape(S, E)
    wqkv = np.asarray(w_qkv, dtype=np.float32)
    wq = np.ascontiguousarray(wqkv[:, 0:E]) * np.float32(SCALE)
    wk = np.ascontiguousarray(wqkv[:, E:2 * E])
    wv = np.ascontiguousarray(wqkv[:, 2 * E:3 * E])
    wout = np.ascontiguousarray(np.asarray(w_out, dtype=np.float32))
    am = np.asarray(allowed_mask).reshape(S, S)
    ident = np.eye(128, dtype=np.float16)

    xT = np.ascontiguousarray(x2.T)  # [E, S]
    in_maps = []
    for c in range(N_CORES):
        lo = c * SQ - HALO
        xTc = np.zeros((E, SK), dtype=np.float32)
        ofs = max(0, -lo)
        xTc[:, ofs:] = xT[:, lo + ofs:c * SQ + SQ]
        pmask = np.zeros((128, NSLICE_COLS), dtype=np.float32)
        for (T, qb, col) in UNITS:
            t0 = lo + T * 128
            if t0 + 128 <= 0:
                continue
            tlo = max(0, -t0)
            s0 = c * SQ + qb * 128
            blk = am[s0:s0 + 128, t0 + tlo:t0 + 128]  # [s, t]
            pmask[tlo:128, col:col + 128] = blk.T.astype(np.float32)
        in_maps.append({
            "xT": xTc.astype(np.float16),
            "wq": wq.astype(np.float16),
            "wk": wk.astype(np.float16),
            "wv": wv.astype(np.float16),
            "wout": wout.astype(np.float16),
            "pmask": pmask.astype(np.float16),
            "ident": ident,
        })
    return in_maps


_CACHED_NC = None


def kernel(x, allowed_mask, w_qkv, w_out):
    global _CACHED_NC
    if _CACHED_NC is None:
        _CACHED_NC = build()
    in_maps = make_in_maps(x, allowed_mask, w_qkv, w_out)
    res = run_bass_kernel_spmd(_CACHED_NC, in_maps, list(range(N_CORES)))
    out = np.concatenate([res.results[c]["out"].astype(np.float32)
                          for c in range(N_CORES)], axis=0)
    return out.reshape(B, S, E)


# revision 45
# speedup vs baseline: 1.8224x; 1.0212x over previous
"""Trainium2 Bass kernel: dense-masked sliding-window attention.

nn.Module semantics (see harness reference):
    B,S,E,H,W = 1, 4096, 1024, 16, 256; D = 64
    qkv = x @ w_qkv -> q,k,v  [B,S,H,D]
    scores = q k^T / sqrt(D), masked to the sliding causal window
             (key j allowed for query i iff i-W < j <= i)
    out = softmax(scores) v  -> [B,S,E] @ w_out

Sharding: sequence-parallel over 8 NeuronCores. Core c owns queries
[512c, 512c+512) and receives a 256-row key/value halo on the left; no
collectives are needed (host concatenates the per-core output rows).

Per-core kernel layout (rev2 -- attention-value swap):
  - x is shipped pre-transposed ([E, 768] feature-major) so every matmul
    contracts over the partition dim without on-device transposes.
  - q^T, k^T are produced transposed ([dims, seq]); v natural
    ([seq, dims]).  PSUM->SBUF copies are spread over ACT and DVE
    (GPSIMD cannot touch PSUM on TRN2).
  - scores are built transposed ([t, s]) in [128, 512] single-bank PSUM
    tiles; softmax skips the max-subtraction (scores are O(1): exp can't
    overflow); the window/causal mask is applied multiplicatively on
    exp(scores) with mask data taken from the allowed_mask input.
  - attention-value products run with exp(scores) STATIONARY and v
    MOVING, so the output lands [q, feat] with queries on partitions:
    the softmax denominators (via 1-column ones-matmuls against the same
    stationary weights) are per-partition values and the whole
    normalization is one reciprocal + one strided multiply per head
    pair.  A PE transpose (identity matmul) restores [feat, q] for the
    output projection.
  - all matmuls run in fp16 (full PE rate).  End-to-end error vs the
    fp32 reference is ~4e-4 scale-relative.
"""

import numpy as np
from contextlib import ExitStack

import concourse.bass as bass
import concourse.tile as tile
from concourse import bacc, mybir
from concourse.bass_utils import run_bass_kernel_spmd

F32 = mybir.dt.float32
F16 = mybir.dt.float16

B, S, E, H, W = 1, 4096, 1024, 16, 256
D = E // H  # 64
SCALE = D ** -0.5
N_CORES = 8
SQ = S // N_CORES          # 512 queries per core
HALO = W                   # 256 halo keys
SK = SQ + HALO             # 768 key rows per core
KC = E // 128              # 8 contraction chunks
QB = SQ // 128             # 4 query blocks per core
TC = SK // 128             # 6 key chunks per core

# Attention band units (T = key chunk, qb = query block), packed into
# three 512-column groups so each score tile is one PSUM bank.  Within a
# group, units sharing a T are contiguous in qb so QK needs one matmul
# per run.  col = offset in the per-head 1536-wide score/mask space.
UNITS = [
    (2, 0, 0), (2, 1, 128), (2, 2, 256), (0, 0, 384),          # block 0
    (1, 0, 512), (1, 1, 640), (4, 2, 768), (4, 3, 896),        # block 1
    (3, 1, 1024), (3, 2, 1152), (3, 3, 1280), (5, 3, 1408),    # block 2
]
# QK matmul runs per block: (T, qb0, nqb, local col0)
QK_RUNS = [
    [(2, 0, 3, 0), (0, 0, 1, 384)],
    [(1, 0, 2, 0), (4, 2, 2, 256)],
    [(3, 1, 3, 0), (5, 3, 1, 384)],
]
NSLICE_COLS = 1536


def _emit_body(ctx: ExitStack, tc_: "tile.TileContext", xT_d, wq_d, wk_d, wv_d,
               wout_d, pmask_d, ident_d, out_d):
    nc = tc_.nc
    P = 128

    xt_pool = ctx.enter_context(tc_.tile_pool(name="xt", bufs=KC))
    # 16-deep: wq+wk stay resident through the interleaved k/QK/v phase;
    # wv reuses wq's early-freed slots, wo reuses wk's (a 10-deep ring
    # deadlocks: wv DMAs would wait on wk slots whose last consumer is
    # behind the v-proj matmuls in the PE queue).
    w_pool = ctx.enter_context(tc_.tile_pool(name="w", bufs=16))
    qt_pool = ctx.enter_context(tc_.tile_pool(name="qt", bufs=KC))
    kt_pool = ctx.enter_context(tc_.tile_pool(name="kt", bufs=KC))
    v_pool = ctx.enter_context(tc_.tile_pool(name="v", bufs=TC))
    pm_pool = ctx.enter_context(tc_.tile_pool(name="pm", bufs=1))
    id_pool = ctx.enter_context(tc_.tile_pool(name="id", bufs=1))
    et_pool = ctx.enter_context(tc_.tile_pool(name="et", bufs=KC))
    at_pool = ctx.enter_context(tc_.tile_pool(name="at", bufs=1))
    atT_pool = ctx.enter_context(tc_.tile_pool(name="atT", bufs=KC))
    rc_pool = ctx.enter_context(tc_.tile_pool(name="rc", bufs=4))
    os_pool = ctx.enter_context(tc_.tile_pool(name="os", bufs=2))
    one_pool = ctx.enter_context(tc_.tile_pool(name="one", bufs=1))
    # PSUM: every working tile is one bank (score/proj/avout/transpose all
    # share the 7-deep "sc" ring); den pins the eighth bank for the whole
    # attention phase.
    sc_pool = ctx.enter_context(tc_.tile_pool(name="sc", bufs=8, space="PSUM"))

    # ---- PE warmup: junk matmuls ramp the PE p-state while DMAs land -----
    junk = one_pool.tile([P, 256], F16, tag="junk")
    nc.vector.memset(junk[:], 0.0)
    jp = sc_pool.tile([P, 256], F32, tag="sc", name="warm")
    for _ in range(14):
        nc.tensor.matmul(jp[:], junk[:, 0:128], junk[:], start=True, stop=True)

    # ---- loads (wq/xt first: they gate the first matmuls) ----------------
    wq, xt = [], []
    for k in range(KC):
        t = w_pool.tile([P, 1024], F16, tag="w", bufs=16)
        nc.sync.dma_start(t[:], wq_d.ap()[k * P:(k + 1) * P, :])
        wq.append(t)
        t = xt_pool.tile([P, SK], F16, tag="xt")
        nc.sync.dma_start(t[:], xT_d.ap()[k * P:(k + 1) * P, :])
        xt.append(t)

    # ---- q^T [E, SQ]: stationary = w_q chunk columns, moving = x^T -------
    qt = []
    for n in range(KC):
        ps = sc_pool.tile([P, 512], F32, tag="sc", name=f"qtp{n}")
        for k in range(KC):
            nc.tensor.matmul(ps[:], wq[k][:, n * P:(n + 1) * P],
                             xt[k][:, HALO:SK], start=(k == 0), stop=(k == KC - 1))
        t = qt_pool.tile([P, SQ], F16, tag="qt")
        nc.scalar.copy(t[:], ps[:])
        qt.append(t)

    # ---- k^T [E, SK] interleaved with attention scores -------------------
    # QK for pair p is emitted after k-proj chunk p+1 so the PE never waits
    # on the copy of kt[p]; exp+mask trail on ACT/DVE.
    wk = []
    for k in range(KC):
        t = w_pool.tile([P, 1024], F16, tag="w", bufs=16)
        nc.sync.dma_start(t[:], wk_d.ap()[k * P:(k + 1) * P, :])
        wk.append(t)
    pm = pm_pool.tile([P, NSLICE_COLS], F16)
    nc.sync.dma_start(pm[:], pmask_d.ap()[:])
    ident = id_pool.tile([P, P], F16)
    nc.sync.dma_start(ident[:], ident_d.ap()[:])
    ones_f = one_pool.tile([P, 1], F32, tag="ones")
    nc.vector.memset(ones_f[:], 1.0)

    kt = []
    et = []

    def emit_kchunk(n):
        psa = sc_pool.tile([P, 512], F32, tag="sc", name=f"ktpa{n}")
        psb = sc_pool.tile([P, 512], F32, tag="sc", name=f"ktpb{n}")
        for k in range(KC):
            nc.tensor.matmul(psa[:], wk[k][:, n * P:(n + 1) * P],
                             xt[k][:, 0:512], start=(k == 0), stop=(k == KC - 1))
            nc.tensor.matmul(psb[:, 0:256], wk[k][:, n * P:(n + 1) * P],
                             xt[k][:, 512:768], start=(k == 0), stop=(k == KC - 1))
        t = kt_pool.tile([P, SK], F16, tag="kt")
        nc.scalar.copy(t[:, 0:512], psa[:])
        nc.vector.tensor_copy(t[:, 512:768], psb[:, 0:256])
        kt.append(t)

    def emit_qk(p):
        # et[p] holds exp(scores) for both heads of pair p: [128, 2, 1536].
        e = et_pool.tile([P, 2 * NSLICE_COLS], F16, tag="et")
        ev = e[:].rearrange("p (s c) -> p s c", s=2)
        for blk in range(3):
            sps = []
            for sub in range(2):
                r0 = 64 * sub
                sp = sc_pool.tile([P, 512], F32, tag="sc")
                sps.append(sp)
                for (T, q0, nq, c0) in QK_RUNS[blk]:
                    nc.tensor.matmul(
                        sp[:, c0:c0 + nq * 128],
                        kt[p][r0:r0 + 64, T * P:(T + 1) * P],
                        qt[p][r0:r0 + 64, q0 * 128:(q0 + nq) * 128],
                        start=True, stop=True, tile_position=(r0, 0))
            for sub in range(2):
                nc.scalar.activation(ev[:, sub, blk * 512:(blk + 1) * 512],
                                     sps[sub][:],
                                     mybir.ActivationFunctionType.Exp)
        nc.vector.tensor_tensor(
            ev[:, :, :], ev[:, :, :],
            pm[:, None, :].broadcast_to([P, 2, NSLICE_COLS]),
            mybir.AluOpType.mult)
        et.append(e)

    # ---- v natural [SK, E] (emitted interleaved below) -------------------
    wv = []
    for k in range(KC):
        t = w_pool.tile([P, 1024], F16, tag="w", bufs=16)
        nc.sync.dma_start(t[:], wv_d.ap()[k * P:(k + 1) * P, :])
        wv.append(t)
    vt = []

    def emit_vchunk(sc):
        # vt rows are [v(64) | 1.0] per head (65 cols): the AV matmul then
        # emits values AND the softmax denominator in one go.
        psa = sc_pool.tile([P, 512], F32, tag="sc", name=f"vpa{sc}")
        psb = sc_pool.tile([P, 512], F32, tag="sc", name=f"vpb{sc}")
        for k in range(KC):
            nc.tensor.matmul(psa[:], xt[k][:, sc * P:(sc + 1) * P],
                             wv[k][:, 0:512], start=(k == 0), stop=(k == KC - 1))
            nc.tensor.matmul(psb[:], xt[k][:, sc * P:(sc + 1) * P],
                             wv[k][:, 512:1024], start=(k == 0), stop=(k == KC - 1))
        t = v_pool.tile([P, H * 65], F16, tag="v")
        tv = t[:].rearrange("p (h c) -> p h c", h=H)
        nc.vector.tensor_copy(
            tv[:, 0:8, 0:64], psa[:].rearrange("p (h c) -> p h c", h=8))
        nc.vector.tensor_copy(
            tv[:, 8:16, 0:64], psb[:].rearrange("p (h c) -> p h c", h=8))
        nc.gpsimd.tensor_copy(
            tv[:, :, 64:65], ones_f[:, None, :].broadcast_to([P, H, 1]))
        vt.append(t)

    # PE order: k-chunks lead their pair's QK by one so the Pool copy of
    # kt[p] is never on the critical path; v-chunks slot in from pair 3 on
    # to keep the PE fed while the exp chain drains on ACT.
    emit_kchunk(0)
    for n in range(1, KC):
        emit_kchunk(n)
        emit_qk(n - 1)
        if n >= 2:
            emit_vchunk(n - 2)
    emit_qk(KC - 1)

    wo = []
    for k in range(KC):
        t = w_pool.tile([P, 1024], F16, tag="w", bufs=16)
        nc.sync.dma_start(t[:], wout_d.ap()[k * P:(k + 1) * P, :])
        wo.append(t)

    # ---- attention values + denominators + normalize ---------------------
    # avout per head: [q, (qb, 65)] with queries on partitions; col 64 of
    # each 65-group is the softmax denominator (from the vt ones column).
    # at layout [q, (pair, qb, sub, d)]: the PE transpose needs each
    # (pair, qb) feature block contiguous (matmul weights APs must have a
    # single free dimension).
    at = at_pool.tile([P, 4096], F16)
    units_by_qb = [[u for u in UNITS if u[1] == qb] for qb in range(QB)]
    atT = []

    def emit_transpose(p):
        psT = sc_pool.tile([P, SQ], F16, tag="sc", name=f"tr{p}")
        for qb in range(QB):
            nc.tensor.transpose(psT[:, qb * P:(qb + 1) * P],
                                at[:, p * 512 + qb * P:p * 512 + (qb + 1) * P],
                                ident[:])
        t = atT_pool.tile([P, SQ], F16, tag="atT")
        if p % 2 == 0:
            nc.scalar.copy(t[:], psT[:])
        else:
            nc.vector.tensor_copy(t[:], psT[:])
        atT.append(t)

    for p in range(KC):
        ev = et[p][:].rearrange("p (s c) -> p s c", s=2)
        avs = []
        for sub in range(2):
            h = 2 * p + sub
            av = sc_pool.tile([P, QB * 65], F32, tag="sc", name=f"av{h}")
            avs.append(av)
            for qb in range(QB):
                us = units_by_qb[qb]
                for i, (T, _, c0) in enumerate(us):
                    nc.tensor.matmul(
                        av[:, qb * 65:qb * 65 + 65],
                        ev[:, sub, c0:c0 + 128],
                        vt[T][:, h * 65:(h + 1) * 65],
                        start=(i == 0), stop=(i == len(us) - 1))
        rc = rc_pool.tile([P, 8], F32, tag="rc")
        at_v = at[:].rearrange("p (pp q s d) -> p pp q s d", pp=KC, q=QB, s=2)
        for sub in range(2):
            nc.vector.reciprocal(
                rc[:, sub * 4:(sub + 1) * 4],
                avs[sub][:].rearrange("p (q c) -> p q c", c=65)[:, :, 64])
        for sub in range(2):
            nc.vector.tensor_tensor(
                at_v[:, p, :, sub, :],
                avs[sub][:].rearrange("p (q c) -> p q c", c=65)[:, :, 0:D],
                rc[:, sub * 4:(sub + 1) * 4][:, :, None]
                    .broadcast_to([P, QB, D]),
                mybir.AluOpType.mult)
    # transpose at [q, f] -> atT [f, q] via PE identity matmuls, batched
    # after the AV loop: the norm chain on DVE drains while the PE runs
    # the remaining AVs, so the transposes rarely wait.
    for p in range(KC):
        emit_transpose(p)

    # ---- output projection ----------------------------------------------
    # Blocks 0..2: one copy per half (ACT||DVE) + one DMA per half (the
    # DMAs overlap later matmuls).  Last block: 256-col column groups so
    # the copy+DMA of each group hides under the next group's matmuls and
    # the serial tail is just the final 256 columns.
    for sb in range(QB - 1):
        psa = sc_pool.tile([P, 512], F32, tag="sc", name=f"opa{sb}")
        psb = sc_pool.tile([P, 512], F32, tag="sc", name=f"opb{sb}")
        for c in range(KC):
            nc.tensor.matmul(psa[:], atT[c][:, sb * P:(sb + 1) * P],
                             wo[c][:, 0:512], start=(c == 0), stop=(c == KC - 1))
            nc.tensor.matmul(psb[:], atT[c][:, sb * P:(sb + 1) * P],
                             wo[c][:, 512:1024], start=(c == 0), stop=(c == KC - 1))
        ob = os_pool.tile([P, E], F16, tag="os")
        nc.scalar.copy(ob[:, 0:512], psa[:])
        nc.vector.tensor_copy(ob[:, 512:1024], psb[:])
        nc.sync.dma_start(out_d.ap()[sb * P:(sb + 1) * P, 0:512],
                          ob[:, 0:512])
        nc.sync.dma_start(out_d.ap()[sb * P:(sb + 1) * P, 512:1024],
                          ob[:, 512:1024])
    sb = QB - 1
    psa = sc_pool.tile([P, 512], F32, tag="sc", name="opa3")
    psb = sc_pool.tile([P, 512], F32, tag="sc", name="opb3")
    ob = os_pool.tile([P, E], F16, tag="os")
    for c in range(KC):
        nc.tensor.matmul(psa[:], atT[c][:, sb * P:(sb + 1) * P],
                         wo[c][:, 0:512], start=(c == 0), stop=(c == KC - 1))
        nc.tensor.matmul(psb[:], atT[c][:, sb * P:(sb + 1) * P],
                         wo[c][:, 512:1024], start=(c == 0), stop=(c == KC - 1))
    nc.scalar.copy(ob[:, 0:512], psa[:])
    nc.vector.tensor_copy(ob[:, 512:1024], psb[:])
    nc.sync.dma_start(out_d.ap()[sb * P:(sb + 1) * P, 0:512], ob[:, 0:512])
    nc.sync.dma_start(out_d.ap()[sb * P:(sb + 1) * P, 512:1024],
                      ob[:, 512:1024])


def build(n_iters: int = 1):
    nc = bacc.Bacc("TRN2", target_bir_lowering=False, debug=False,
                   num_devices=N_CORES)
    xT_d = nc.dram_tensor("xT", [E, SK], F16, kind="ExternalInput")
    wq_d = nc.dram_tensor("wq", [E, E], F16, kind="ExternalInput")
    wk_d = nc.dram_tensor("wk", [E, E], F16, kind="ExternalInput")
    wv_d = nc.dram_tensor("wv", [E, E], F16, kind="ExternalInput")
    wout_d = nc.dram_tensor("wout", [E, E], F16, kind="ExternalInput")
    pmask_d = nc.dram_tensor("pmask", [128, NSLICE_COLS], F16,
                             kind="ExternalInput")
    ident_d = nc.dram_tensor("ident", [128, 128], F16, kind="ExternalInput")
    out_d = nc.dram_tensor("out", [SQ, E], F16, kind="ExternalOutput")
    with tile.TileContext(nc) as tc_, ExitStack() as ctx:
        if n_iters > 1:
            with tc_.For_i(0, n_iters, 1):
                _emit_body(ctx, tc_, xT_d, wq_d, wk_d, wv_d, wout_d, pmask_d,
                           ident_d, out_d)
        else:
            _emit_body(ctx, tc_, xT_d, wq_d, wk_d, wv_d, wout_d, pmask_d,
                       ident_d, out_d)
    nc.compile()
    return nc


def make_in_maps(x, allowed_mask, w_qkv, w_out):
    """Shard the full inputs into per-core input maps (host marshaling)."""
    x2 = np.asarray(x, dtype=np.float32).reshape(S, E)
    wqkv = np.asarray(w_qkv, dtype=np.float32)
    wq = np.ascontiguousarray(wqkv[:, 0:E]) * np.float32(SCALE)
    wk = np.ascontiguousarray(wqkv[:, E:2 * E])
    wv = np.ascontiguousarray(wqkv[:, 2 * E:3 * E])
    wout = np.ascontiguousarray(np.asarray(w_out, dtype=np.float32))
    am = np.asarray(allowed_mask).reshape(S, S)
    ident = np.eye(128, dtype=np.float16)

    xT = np.ascontiguousarray(x2.T)  # [E, S]
    in_maps = []
    for c in range(N_CORES):
        lo = c * SQ - HALO
        xTc = np.zeros((E, SK), dtype=np.float32)
        ofs = max(0, -lo)
        xTc[:, ofs:] = xT[:, lo + ofs:c * SQ + SQ]
        pmask = np.zeros((128, NSLICE_COLS), dtype=np.float32)
        for (T, qb, col) in UNITS:
            t0 = lo + T * 128
            if t0 + 128 <= 0:
                continue
            tlo = max(0, -t0)
            s0 = c * SQ + qb * 128
            blk = am[s0:s0 + 128, t0 + tlo:t0 + 128]  # [s, t]
            pmask[tlo:128, col:col + 128] = blk.T.astype(np.float32)
        in_maps.append({
            "xT": xTc.astype(np.float16),
            "wq": wq.astype(np.float16),
            "wk": wk.astype(np.float16),
            "wv": wv.astype(np.float16),
            "wout": wout.astype(np.float16),
            "pmask": pmask.astype(np.float16),
            "ident": ident,
        })
    return in_maps


_CACHED_NC = None


def kernel(x, allowed_mask, w_qkv, w_out):
    global _CACHED_NC
    if _CACHED_NC is None:
        _CACHED_NC = build()
    in_maps = make_in_maps(x, allowed_mask, w_qkv, w_out)
    res = run_bass_kernel_spmd(_CACHED_NC, in_maps, list(range(N_CORES)))
    out = np.concatenate([res.results[c]["out"].astype(np.float32)
                          for c in range(N_CORES)], axis=0)
    return out.reshape(B, S, E)


# revision 46
# speedup vs baseline: 1.8291x; 1.0036x over previous
"""Trainium2 Bass kernel: dense-masked sliding-window attention.

nn.Module semantics (see harness reference):
    B,S,E,H,W = 1, 4096, 1024, 16, 256; D = 64
    qkv = x @ w_qkv -> q,k,v  [B,S,H,D]
    scores = q k^T / sqrt(D), masked to the sliding causal window
             (key j allowed for query i iff i-W < j <= i)
    out = softmax(scores) v  -> [B,S,E] @ w_out

Sharding: sequence-parallel over 8 NeuronCores. Core c owns queries
[512c, 512c+512) and receives a 256-row key/value halo on the left; no
collectives are needed (host concatenates the per-core output rows).

Per-core kernel layout (rev2 -- attention-value swap):
  - x is shipped pre-transposed ([E, 768] feature-major) so every matmul
    contracts over the partition dim without on-device transposes.
  - q^T, k^T are produced transposed ([dims, seq]); v natural
    ([seq, dims]).  PSUM->SBUF copies are spread over ACT and DVE
    (GPSIMD cannot touch PSUM on TRN2).
  - scores are built transposed ([t, s]) in [128, 512] single-bank PSUM
    tiles; softmax skips the max-subtraction (scores are O(1): exp can't
    overflow); the window/causal mask is applied multiplicatively on
    exp(scores) with mask data taken from the allowed_mask input.
  - attention-value products run with exp(scores) STATIONARY and v
    MOVING, so the output lands [q, feat] with queries on partitions:
    the softmax denominators (via 1-column ones-matmuls against the same
    stationary weights) are per-partition values and the whole
    normalization is one reciprocal + one strided multiply per head
    pair.  A PE transpose (identity matmul) restores [feat, q] for the
    output projection.
  - all matmuls run in fp16 (full PE rate).  End-to-end error vs the
    fp32 reference is ~4e-4 scale-relative.
"""

import numpy as np
from contextlib import ExitStack

import concourse.bass as bass
import concourse.tile as tile
from concourse import bacc, mybir
from concourse.bass_utils import run_bass_kernel_spmd

F32 = mybir.dt.float32
F16 = mybir.dt.float16

B, S, E, H, W = 1, 4096, 1024, 16, 256
D = E // H  # 64
SCALE = D ** -0.5
N_CORES = 8
SQ = S // N_CORES          # 512 queries per core
HALO = W                   # 256 halo keys
SK = SQ + HALO             # 768 key rows per core
KC = E // 128              # 8 contraction chunks
QB = SQ // 128             # 4 query blocks per core
TC = SK // 128             # 6 key chunks per core

# Attention band units (T = key chunk, qb = query block), packed into
# three 512-column groups so each score tile is one PSUM bank.  Within a
# group, units sharing a T are contiguous in qb so QK needs one matmul
# per run.  col = offset in the per-head 1536-wide score/mask space.
UNITS = [
    (2, 0, 0), (2, 1, 128), (2, 2, 256), (0, 0, 384),          # block 0
    (1, 0, 512), (1, 1, 640), (4, 2, 768), (4, 3, 896),        # block 1
    (3, 1, 1024), (3, 2, 1152), (3, 3, 1280), (5, 3, 1408),    # block 2
]
# QK matmul runs per block: (T, qb0, nqb, local col0)
QK_RUNS = [
    [(2, 0, 3, 0), (0, 0, 1, 384)],
    [(1, 0, 2, 0), (4, 2, 2, 256)],
    [(3, 1, 3, 0), (5, 3, 1, 384)],
]
NSLICE_COLS = 1536


def _emit_body(ctx: ExitStack, tc_: "tile.TileContext", xT_d, wq_d, wk_d, wv_d,
               wout_d, pmask_d, ident_d, out_d, warm=True):
    nc = tc_.nc
    P = 128

    xt_pool = ctx.enter_context(tc_.tile_pool(name="xt", bufs=KC))
    # 16-deep: wq+wk stay resident through the interleaved k/QK/v phase;
    # wv reuses wq's early-freed slots, wo reuses wk's (a 10-deep ring
    # deadlocks: wv DMAs would wait on wk slots whose last consumer is
    # behind the v-proj matmuls in the PE queue).
    w_pool = ctx.enter_context(tc_.tile_pool(name="w", bufs=16))
    qt_pool = ctx.enter_context(tc_.tile_pool(name="qt", bufs=KC))
    kt_pool = ctx.enter_context(tc_.tile_pool(name="kt", bufs=KC))
    v_pool = ctx.enter_context(tc_.tile_pool(name="v", bufs=TC))
    pm_pool = ctx.enter_context(tc_.tile_pool(name="pm", bufs=1))
    id_pool = ctx.enter_context(tc_.tile_pool(name="id", bufs=1))
    et_pool = ctx.enter_context(tc_.tile_pool(name="et", bufs=KC))
    at_pool = ctx.enter_context(tc_.tile_pool(name="at", bufs=1))
    atT_pool = ctx.enter_context(tc_.tile_pool(name="atT", bufs=KC))
    rc_pool = ctx.enter_context(tc_.tile_pool(name="rc", bufs=4))
    os_pool = ctx.enter_context(tc_.tile_pool(name="os", bufs=2))
    one_pool = ctx.enter_context(tc_.tile_pool(name="one", bufs=1))
    # PSUM: every working tile is one bank (score/proj/avout/transpose all
    # share the 7-deep "sc" ring); den pins the eighth bank for the whole
    # attention phase.
    sc_pool = ctx.enter_context(tc_.tile_pool(name="sc", bufs=8, space="PSUM"))

    # ---- PE warmup: junk matmuls ramp the PE p-state while the first
    # DMAs land.  Only worth it on a cold start: in a timing loop the PE
    # stays hot across iterations and the junk would add ~3us/iter.
    if warm:
        junk = one_pool.tile([P, 256], F16, tag="junk")
        nc.vector.memset(junk[:], 0.0)
        jp = sc_pool.tile([P, 256], F32, tag="sc", name="warm")
        for _ in range(14):
            nc.tensor.matmul(jp[:], junk[:, 0:128], junk[:],
                             start=True, stop=True)

    # ---- loads (wq/xt first: they gate the first matmuls) ----------------
    wq, xt = [], []
    for k in range(KC):
        t = w_pool.tile([P, 1024], F16, tag="w", bufs=16)
        nc.sync.dma_start(t[:], wq_d.ap()[k * P:(k + 1) * P, :])
        wq.append(t)
        t = xt_pool.tile([P, SK], F16, tag="xt")
        nc.sync.dma_start(t[:], xT_d.ap()[k * P:(k + 1) * P, :])
        xt.append(t)

    # ---- q^T [E, SQ]: stationary = w_q chunk columns, moving = x^T -------
    qt = []
    for n in range(KC):
        ps = sc_pool.tile([P, 512], F32, tag="sc", name=f"qtp{n}")
        for k in range(KC):
            nc.tensor.matmul(ps[:], wq[k][:, n * P:(n + 1) * P],
                             xt[k][:, HALO:SK], start=(k == 0), stop=(k == KC - 1))
        t = qt_pool.tile([P, SQ], F16, tag="qt")
        nc.scalar.copy(t[:], ps[:])
        qt.append(t)

    # ---- k^T [E, SK] interleaved with attention scores -------------------
    # QK for pair p is emitted after k-proj chunk p+1 so the PE never waits
    # on the copy of kt[p]; exp+mask trail on ACT/DVE.
    wk = []
    for k in range(KC):
        t = w_pool.tile([P, 1024], F16, tag="w", bufs=16)
        nc.sync.dma_start(t[:], wk_d.ap()[k * P:(k + 1) * P, :])
        wk.append(t)
    pm = pm_pool.tile([P, NSLICE_COLS], F16)
    nc.sync.dma_start(pm[:], pmask_d.ap()[:])
    ident = id_pool.tile([P, P], F16)
    nc.sync.dma_start(ident[:], ident_d.ap()[:])
    ones_f = one_pool.tile([P, 1], F32, tag="ones")
    nc.vector.memset(ones_f[:], 1.0)

    kt = []
    et = []

    def emit_kchunk(n):
        psa = sc_pool.tile([P, 512], F32, tag="sc", name=f"ktpa{n}")
        psb = sc_pool.tile([P, 512], F32, tag="sc", name=f"ktpb{n}")
        for k in range(KC):
            nc.tensor.matmul(psa[:], wk[k][:, n * P:(n + 1) * P],
                             xt[k][:, 0:512], start=(k == 0), stop=(k == KC - 1))
            nc.tensor.matmul(psb[:, 0:256], wk[k][:, n * P:(n + 1) * P],
                             xt[k][:, 512:768], start=(k == 0), stop=(k == KC - 1))
        t = kt_pool.tile([P, SK], F16, tag="kt")
        nc.scalar.copy(t[:, 0:512], psa[:])
        nc.vector.tensor_copy(t[:, 512:768], psb[:, 0:256])
        kt.append(t)

    def emit_qk(p):
        # et[p] holds exp(scores) for both heads of pair p: [128, 2, 1536].
        e = et_pool.tile([P, 2 * NSLICE_COLS], F16, tag="et")
        ev = e[:].rearrange("p (s c) -> p s c", s=2)
        for blk in range(3):
            sps = []
            for sub in range(2):
                r0 = 64 * sub
                sp = sc_pool.tile([P, 512], F32, tag="sc")
                sps.append(sp)
                for (T, q0, nq, c0) in QK_RUNS[blk]:
                    nc.tensor.matmul(
                        sp[:, c0:c0 + nq * 128],
                        kt[p][r0:r0 + 64, T * P:(T + 1) * P],
                        qt[p][r0:r0 + 64, q0 * 128:(q0 + nq) * 128],
                        start=True, stop=True, tile_position=(r0, 0))
            for sub in range(2):
                nc.scalar.activation(ev[:, sub, blk * 512:(blk + 1) * 512],
                                     sps[sub][:],
                                     mybir.ActivationFunctionType.Exp)
        nc.vector.tensor_tensor(
            ev[:, :, :], ev[:, :, :],
            pm[:, None, :].broadcast_to([P, 2, NSLICE_COLS]),
            mybir.AluOpType.mult)
        et.append(e)

    # ---- v natural [SK, E] (emitted interleaved below) -------------------
    wv = []
    for k in range(KC):
        t = w_pool.tile([P, 1024], F16, tag="w", bufs=16)
        nc.sync.dma_start(t[:], wv_d.ap()[k * P:(k + 1) * P, :])
        wv.append(t)
    vt = []

    def emit_vchunk(sc):
        # vt rows are [v(64) | 1.0] per head (65 cols): the AV matmul then
        # emits values AND the softmax denominator in one go.
        psa = sc_pool.tile([P, 512], F32, tag="sc", name=f"vpa{sc}")
        psb = sc_pool.tile([P, 512], F32, tag="sc", name=f"vpb{sc}")
        for k in range(KC):
            nc.tensor.matmul(psa[:], xt[k][:, sc * P:(sc + 1) * P],
                             wv[k][:, 0:512], start=(k == 0), stop=(k == KC - 1))
            nc.tensor.matmul(psb[:], xt[k][:, sc * P:(sc + 1) * P],
                             wv[k][:, 512:1024], start=(k == 0), stop=(k == KC - 1))
        t = v_pool.tile([P, H * 65], F16, tag="v")
        tv = t[:].rearrange("p (h c) -> p h c", h=H)
        nc.vector.tensor_copy(
            tv[:, 0:8, 0:64], psa[:].rearrange("p (h c) -> p h c", h=8))
        nc.vector.tensor_copy(
            tv[:, 8:16, 0:64], psb[:].rearrange("p (h c) -> p h c", h=8))
        nc.gpsimd.tensor_copy(
            tv[:, :, 64:65], ones_f[:, None, :].broadcast_to([P, H, 1]))
        vt.append(t)

    # PE order: k-chunks lead their pair's QK by one so the Pool copy of
    # kt[p] is never on the critical path; v-chunks slot in from pair 3 on
    # to keep the PE fed while the exp chain drains on ACT.
    emit_kchunk(0)
    for n in range(1, KC):
        emit_kchunk(n)
        emit_qk(n - 1)
        if n >= 2:
            emit_vchunk(n - 2)
    emit_qk(KC - 1)

    wo = []
    for k in range(KC):
        t = w_pool.tile([P, 1024], F16, tag="w", bufs=16)
        nc.sync.dma_start(t[:], wout_d.ap()[k * P:(k + 1) * P, :])
        wo.append(t)

    # ---- attention values + denominators + normalize ---------------------
    # avout per head: [q, (qb, 65)] with queries on partitions; col 64 of
    # each 65-group is the softmax denominator (from the vt ones column).
    # at layout [q, (pair, qb, sub, d)]: the PE transpose needs each
    # (pair, qb) feature block contiguous (matmul weights APs must have a
    # single free dimension).
    at = at_pool.tile([P, 4096], F16)
    units_by_qb = [[u for u in UNITS if u[1] == qb] for qb in range(QB)]
    atT = []

    def emit_transpose(p):
        psT = sc_pool.tile([P, SQ], F16, tag="sc", name=f"tr{p}")
        for qb in range(QB):
            nc.tensor.transpose(psT[:, qb * P:(qb + 1) * P],
                                at[:, p * 512 + qb * P:p * 512 + (qb + 1) * P],
                                ident[:])
        t = atT_pool.tile([P, SQ], F16, tag="atT")
        if p % 2 == 0:
            nc.scalar.copy(t[:], psT[:])
        else:
            nc.vector.tensor_copy(t[:], psT[:])
        atT.append(t)

    for p in range(KC):
        ev = et[p][:].rearrange("p (s c) -> p s c", s=2)
        avs = []
        for sub in range(2):
            h = 2 * p + sub
            av = sc_pool.tile([P, QB * 65], F32, tag="sc", name=f"av{h}")
            avs.append(av)
            for qb in range(QB):
                us = units_by_qb[qb]
                for i, (T, _, c0) in enumerate(us):
                    nc.tensor.matmul(
                        av[:, qb * 65:qb * 65 + 65],
                        ev[:, sub, c0:c0 + 128],
                        vt[T][:, h * 65:(h + 1) * 65],
                        start=(i == 0), stop=(i == len(us) - 1))
        rc = rc_pool.tile([P, 8], F32, tag="rc")
        at_v = at[:].rearrange("p (pp q s d) -> p pp q s d", pp=KC, q=QB, s=2)
        for sub in range(2):
            nc.vector.reciprocal(
                rc[:, sub * 4:(sub + 1) * 4],
                avs[sub][:].rearrange("p (q c) -> p q c", c=65)[:, :, 64])
        for sub in range(2):
            nc.vector.tensor_tensor(
                at_v[:, p, :, sub, :],
                avs[sub][:].rearrange("p (q c) -> p q c", c=65)[:, :, 0:D],
                rc[:, sub * 4:(sub + 1) * 4][:, :, None]
                    .broadcast_to([P, QB, D]),
                mybir.AluOpType.mult)
    # transpose at [q, f] -> atT [f, q] via PE identity matmuls, batched
    # after the AV loop: the norm chain on DVE drains while the PE runs
    # the remaining AVs, so the transposes rarely wait.
    for p in range(KC):
        emit_transpose(p)

    # ---- output projection ----------------------------------------------
    # Blocks 0..2: one copy per half (ACT||DVE) + one DMA per half (the
    # DMAs overlap later matmuls).  Last block: 256-col column groups so
    # the copy+DMA of each group hides under the next group's matmuls and
    # the serial tail is just the final 256 columns.
    for sb in range(QB - 1):
        psa = sc_pool.tile([P, 512], F32, tag="sc", name=f"opa{sb}")
        psb = sc_pool.tile([P, 512], F32, tag="sc", name=f"opb{sb}")
        for c in range(KC):
            nc.tensor.matmul(psa[:], atT[c][:, sb * P:(sb + 1) * P],
                             wo[c][:, 0:512], start=(c == 0), stop=(c == KC - 1))
            nc.tensor.matmul(psb[:], atT[c][:, sb * P:(sb + 1) * P],
                             wo[c][:, 512:1024], start=(c == 0), stop=(c == KC - 1))
        ob = os_pool.tile([P, E], F16, tag="os")
        nc.scalar.copy(ob[:, 0:512], psa[:])
        nc.vector.tensor_copy(ob[:, 512:1024], psb[:])
        nc.sync.dma_start(out_d.ap()[sb * P:(sb + 1) * P, 0:512],
                          ob[:, 0:512])
        nc.sync.dma_start(out_d.ap()[sb * P:(sb + 1) * P, 512:1024],
                          ob[:, 512:1024])
    sb = QB - 1
    psa = sc_pool.tile([P, 512], F32, tag="sc", name="opa3")
    psb = sc_pool.tile([P, 512], F32, tag="sc", name="opb3")
    ob = os_pool.tile([P, E], F16, tag="os")
    for c in range(KC):
        nc.tensor.matmul(psa[:], atT[c][:, sb * P:(sb + 1) * P],
                         wo[c][:, 0:512], start=(c == 0), stop=(c == KC - 1))
        nc.tensor.matmul(psb[:], atT[c][:, sb * P:(sb + 1) * P],
                         wo[c][:, 512:1024], start=(c == 0), stop=(c == KC - 1))
    nc.scalar.copy(ob[:, 0:512], psa[:])
    nc.vector.tensor_copy(ob[:, 512:1024], psb[:])
    nc.sync.dma_start(out_d.ap()[sb * P:(sb + 1) * P, 0:512], ob[:, 0:512])
    nc.sync.dma_start(out_d.ap()[sb * P:(sb + 1) * P, 512:1024],
                      ob[:, 512:1024])


def build(n_iters: int = 1):
    nc = bacc.Bacc("TRN2", target_bir_lowering=False, debug=False,
                   num_devices=N_CORES)
    xT_d = nc.dram_tensor("xT", [E, SK], F16, kind="ExternalInput")
    wq_d = nc.dram_tensor("wq", [E, E], F16, kind="ExternalInput")
    wk_d = nc.dram_tensor("wk", [E, E], F16, kind="ExternalInput")
    wv_d = nc.dram_tensor("wv", [E, E], F16, kind="ExternalInput")
    wout_d = nc.dram_tensor("wout", [E, E], F16, kind="ExternalInput")
    pmask_d = nc.dram_tensor("pmask", [128, NSLICE_COLS], F16,
                             kind="ExternalInput")
    ident_d = nc.dram_tensor("ident", [128, 128], F16, kind="ExternalInput")
    out_d = nc.dram_tensor("out", [SQ, E], F16, kind="ExternalOutput")
    with tile.TileContext(nc) as tc_, ExitStack() as ctx:
        if n_iters > 1:
            with tc_.For_i(0, n_iters, 1):
                _emit_body(ctx, tc_, xT_d, wq_d, wk_d, wv_d, wout_d, pmask_d,
                           ident_d, out_d, warm=False)
        else:
            _emit_body(ctx, tc_, xT_d, wq_d, wk_d, wv_d, wout_d, pmask_d,
                       ident_d, out_d, warm=True)
    nc.compile()
    return nc


def make_in_maps(x, allowed_mask, w_qkv, w_out):
    """Shard the full inputs into per-core input maps (host marshaling)."""
    x2 = np.asarray(x, dtype=np.float32).reshape(S, E)
    wqkv = np.asarray(w_qkv, dtype=np.float32)
    wq = np.ascontiguousarray(wqkv[:, 0:E]) * np.float32(SCALE)
    wk = np.ascontiguousarray(wqkv[:, E:2 * E])
    wv = np.ascontiguousarray(wqkv[:, 2 * E:3 * E])
    wout = np.ascontiguousarray(np.asarray(w_out, dtype=np.float32))
    am = np.asarray(allowed_mask).reshape(S, S)
    ident = np.eye(128, dtype=np.float16)

    xT = np.ascontiguousarray(x2.T)  # [E, S]
    in_maps = []
    for c in range(N_CORES):
        lo = c * SQ - HALO
        xTc = np.zeros((E, SK), dtype=np.float32)
        ofs = max(0, -lo)
        xTc[:, ofs:] = xT[:, lo + ofs:c * SQ + SQ]
        pmask = np.zeros((128, NSLICE_COLS), dtype=np.float32)
        for (T, qb, col) in UNITS:
            t0 = lo + T * 128
            if t0 + 128 <= 0:
                continue
            tlo = max(0, -t0)
            s0 = c * SQ + qb * 128
            blk = am[s0:s0 + 128, t0 + tlo:t0 + 128]  # [s, t]
            pmask[tlo:128, col:col + 128] = blk.T.astype(np.float32)
        in_maps.append({
            "xT": xTc.astype(np.float16),
            "wq": wq.astype(np.float16),
            "wk": wk.astype(np.float16),
            "wv": wv.astype(np.float16),
            "wout": wout.astype(np.float16),
            "pmask": pmask.astype(np.float16),
            "ident": ident,
        })
    return in_maps


_CACHED_NC = None


def kernel(x, allowed_mask, w_qkv, w_out):
    global _CACHED_NC
    if _CACHED_NC is None:
        _CACHED_NC = build()
    in_maps = make_in_maps(x, allowed_mask, w_qkv, w_out)
    res = run_bass_kernel_spmd(_CACHED_NC, in_maps, list(range(N_CORES)))
    out = np.concatenate([res.results[c]["out"].astype(np.float32)
                          for c in range(N_CORES)], axis=0)
    return out.reshape(B, S, E)


# revision 49
# speedup vs baseline: 1.8649x; 1.0196x over previous
"""Trainium2 Bass kernel: dense-masked sliding-window attention.

nn.Module semantics (see harness reference):
    B,S,E,H,W = 1, 4096, 1024, 16, 256; D = 64
    qkv = x @ w_qkv -> q,k,v  [B,S,H,D]
    scores = q k^T / sqrt(D), masked to the sliding causal window
             (key j allowed for query i iff i-W < j <= i)
    out = softmax(scores) v  -> [B,S,E] @ w_out

Sharding: sequence-parallel over 8 NeuronCores. Core c owns queries
[512c, 512c+512) and receives a 256-row key/value halo on the left; no
collectives are needed (host concatenates the per-core output rows).

Per-core kernel layout (rev2 -- attention-value swap):
  - x is shipped pre-transposed ([E, 768] feature-major) so every matmul
    contracts over the partition dim without on-device transposes.
  - q^T, k^T are produced transposed ([dims, seq]); v natural
    ([seq, dims]).  PSUM->SBUF copies are spread over ACT and DVE
    (GPSIMD cannot touch PSUM on TRN2).
  - scores are built transposed ([t, s]) in [128, 512] single-bank PSUM
    tiles; softmax skips the max-subtraction (scores are O(1): exp can't
    overflow); the window/causal mask is applied multiplicatively on
    exp(scores) with mask data taken from the allowed_mask input.
  - attention-value products run with exp(scores) STATIONARY and v
    MOVING, so the output lands [q, feat] with queries on partitions:
    the softmax denominators (via 1-column ones-matmuls against the same
    stationary weights) are per-partition values and the whole
    normalization is one reciprocal + one strided multiply per head
    pair.  A PE transpose (identity matmul) restores [feat, q] for the
    output projection.
  - all matmuls run in fp16 (full PE rate).  End-to-end error vs the
    fp32 reference is ~4e-4 scale-relative.
"""

import numpy as np
from contextlib import ExitStack

import concourse.bass as bass
import concourse.tile as tile
from concourse import bacc, mybir
from concourse.bass_utils import run_bass_kernel_spmd

F32 = mybir.dt.float32
F16 = mybir.dt.float16

B, S, E, H, W = 1, 4096, 1024, 16, 256
D = E // H  # 64
SCALE = D ** -0.5
N_CORES = 8
SQ = S // N_CORES          # 512 queries per core
HALO = W                   # 256 halo keys
SK = SQ + HALO             # 768 key rows per core
KC = E // 128              # 8 contraction chunks
QB = SQ // 128             # 4 query blocks per core
TC = SK // 128             # 6 key chunks per core

# Attention band units (T = key chunk, qb = query block), packed into
# three 512-column groups so each score tile is one PSUM bank.  Within a
# group, units sharing a T are contiguous in qb so QK needs one matmul
# per run.  col = offset in the per-head 1536-wide score/mask space.
UNITS = [
    (2, 0, 0), (2, 1, 128), (2, 2, 256), (0, 0, 384),          # block 0
    (1, 0, 512), (1, 1, 640), (4, 2, 768), (4, 3, 896),        # block 1
    (3, 1, 1024), (3, 2, 1152), (3, 3, 1280), (5, 3, 1408),    # block 2
]
# QK matmul runs per block: (T, qb0, nqb, local col0)
QK_RUNS = [
    [(2, 0, 3, 0), (0, 0, 1, 384)],
    [(1, 0, 2, 0), (4, 2, 2, 256)],
    [(3, 1, 3, 0), (5, 3, 1, 384)],
]
NSLICE_COLS = 1536


def _emit_body(ctx: ExitStack, tc_: "tile.TileContext", xT_d, wq_d, wk_d, wv_d,
               wout_d, pmask_d, ident_d, out_d, warm=True):
    nc = tc_.nc
    P = 128

    xt_pool = ctx.enter_context(tc_.tile_pool(name="xt", bufs=KC))
    # 16-deep: wq+wk stay resident through the interleaved k/QK/v phase;
    # wv reuses wq's early-freed slots, wo reuses wk's (a 10-deep ring
    # deadlocks: wv DMAs would wait on wk slots whose last consumer is
    # behind the v-proj matmuls in the PE queue).
    w_pool = ctx.enter_context(tc_.tile_pool(name="w", bufs=16))
    qt_pool = ctx.enter_context(tc_.tile_pool(name="qt", bufs=KC))
    kt_pool = ctx.enter_context(tc_.tile_pool(name="kt", bufs=KC))
    v_pool = ctx.enter_context(tc_.tile_pool(name="v", bufs=TC))
    pm_pool = ctx.enter_context(tc_.tile_pool(name="pm", bufs=1))
    id_pool = ctx.enter_context(tc_.tile_pool(name="id", bufs=1))
    et_pool = ctx.enter_context(tc_.tile_pool(name="et", bufs=KC))
    at_pool = ctx.enter_context(tc_.tile_pool(name="at", bufs=1))
    atT_pool = ctx.enter_context(tc_.tile_pool(name="atT", bufs=KC))
    rc_pool = ctx.enter_context(tc_.tile_pool(name="rc", bufs=4))
    os_pool = ctx.enter_context(tc_.tile_pool(name="os", bufs=2))
    one_pool = ctx.enter_context(tc_.tile_pool(name="one", bufs=1))
    # PSUM: every working tile is one bank (score/proj/avout/transpose all
    # share the 7-deep "sc" ring); den pins the eighth bank for the whole
    # attention phase.
    sc_pool = ctx.enter_context(tc_.tile_pool(name="sc", bufs=8, space="PSUM"))

    # ---- PE warmup: junk matmuls ramp the PE p-state while the first
    # DMAs land.  Only worth it on a cold start: in a timing loop the PE
    # stays hot across iterations and the junk would add ~3us/iter.
    if warm:
        junk = one_pool.tile([P, 256], F16, tag="junk")
        nc.vector.memset(junk[:], 0.0)
        jp = sc_pool.tile([P, 256], F32, tag="sc", name="warm")
        for _ in range(14):
            nc.tensor.matmul(jp[:], junk[:, 0:128], junk[:],
                             start=True, stop=True)

    # ---- loads (wq/xt first: they gate the first matmuls) ----------------
    wq, xt = [], []
    for k in range(KC):
        t = w_pool.tile([P, 1024], F16, tag="w", bufs=16)
        nc.sync.dma_start(t[:], wq_d.ap()[k * P:(k + 1) * P, :])
        wq.append(t)
        t = xt_pool.tile([P, SK], F16, tag="xt")
        nc.sync.dma_start(t[:], xT_d.ap()[k * P:(k + 1) * P, :])
        xt.append(t)

    # ---- q^T [E, SQ]: stationary = w_q chunk columns, moving = x^T -------
    qt = []
    for n in range(KC):
        ps = sc_pool.tile([P, 512], F32, tag="sc", name=f"qtp{n}")
        for k in range(KC):
            nc.tensor.matmul(ps[:], wq[k][:, n * P:(n + 1) * P],
                             xt[k][:, HALO:SK], start=(k == 0), stop=(k == KC - 1))
        t = qt_pool.tile([P, SQ], F16, tag="qt")
        nc.scalar.copy(t[:], ps[:])
        qt.append(t)

    # ---- k^T [E, SK] interleaved with attention scores -------------------
    # QK for pair p is emitted after k-proj chunk p+1 so the PE never waits
    # on the copy of kt[p]; exp+mask trail on ACT/DVE.
    wk = []
    for k in range(KC):
        t = w_pool.tile([P, 1024], F16, tag="w", bufs=16)
        nc.sync.dma_start(t[:], wk_d.ap()[k * P:(k + 1) * P, :])
        wk.append(t)
    pm = pm_pool.tile([P, NSLICE_COLS], F16)
    nc.sync.dma_start(pm[:], pmask_d.ap()[:])
    ident = id_pool.tile([P, P], F16)
    nc.sync.dma_start(ident[:], ident_d.ap()[:])
    ones_f = one_pool.tile([P, 1], F32, tag="ones")
    nc.vector.memset(ones_f[:], 1.0)

    kt = []
    et = []

    def emit_kchunk(n):
        psa = sc_pool.tile([P, 512], F32, tag="sc", name=f"ktpa{n}")
        psb = sc_pool.tile([P, 512], F32, tag="sc", name=f"ktpb{n}")
        for k in range(KC):
            nc.tensor.matmul(psa[:], wk[k][:, n * P:(n + 1) * P],
                             xt[k][:, 0:512], start=(k == 0), stop=(k == KC - 1))
            nc.tensor.matmul(psb[:, 0:256], wk[k][:, n * P:(n + 1) * P],
                             xt[k][:, 512:768], start=(k == 0), stop=(k == KC - 1))
        t = kt_pool.tile([P, SK], F16, tag="kt")
        nc.scalar.copy(t[:, 0:512], psa[:])
        nc.vector.tensor_copy(t[:, 512:768], psb[:, 0:256])
        kt.append(t)

    def emit_qk(p):
        # et[p] holds exp(scores) for both heads of pair p: [128, 2, 1536].
        e = et_pool.tile([P, 2 * NSLICE_COLS], F16, tag="et")
        ev = e[:].rearrange("p (s c) -> p s c", s=2)
        for blk in range(3):
            sps = []
            for sub in range(2):
                r0 = 64 * sub
                sp = sc_pool.tile([P, 512], F32, tag="sc")
                sps.append(sp)
                for (T, q0, nq, c0) in QK_RUNS[blk]:
                    nc.tensor.matmul(
                        sp[:, c0:c0 + nq * 128],
                        kt[p][r0:r0 + 64, T * P:(T + 1) * P],
                        qt[p][r0:r0 + 64, q0 * 128:(q0 + nq) * 128],
                        start=True, stop=True, tile_position=(r0, 0))
            for sub in range(2):
                nc.scalar.activation(ev[:, sub, blk * 512:(blk + 1) * 512],
                                     sps[sub][:],
                                     mybir.ActivationFunctionType.Exp)
        nc.vector.tensor_tensor(
            ev[:, :, :], ev[:, :, :],
            pm[:, None, :].broadcast_to([P, 2, NSLICE_COLS]),
            mybir.AluOpType.mult)
        et.append(e)

    # ---- v natural [SK, E] (emitted interleaved below) -------------------
    wv = []
    for k in range(KC):
        t = w_pool.tile([P, 1024], F16, tag="w", bufs=16)
        nc.sync.dma_start(t[:], wv_d.ap()[k * P:(k + 1) * P, :])
        wv.append(t)
    vt = []

    def emit_vchunk(sc):
        # vt rows are [v(64) | 1.0] per head (65 cols): the AV matmul then
        # emits values AND the softmax denominator in one go.
        psa = sc_pool.tile([P, 512], F32, tag="sc", name=f"vpa{sc}")
        psb = sc_pool.tile([P, 512], F32, tag="sc", name=f"vpb{sc}")
        for k in range(KC):
            nc.tensor.matmul(psa[:], xt[k][:, sc * P:(sc + 1) * P],
                             wv[k][:, 0:512], start=(k == 0), stop=(k == KC - 1))
            nc.tensor.matmul(psb[:], xt[k][:, sc * P:(sc + 1) * P],
                             wv[k][:, 512:1024], start=(k == 0), stop=(k == KC - 1))
        t = v_pool.tile([P, H * 65], F16, tag="v")
        tv = t[:].rearrange("p (h c) -> p h c", h=H)
        nc.vector.tensor_copy(
            tv[:, 0:8, 0:64], psa[:].rearrange("p (h c) -> p h c", h=8))
        nc.vector.tensor_copy(
            tv[:, 8:16, 0:64], psb[:].rearrange("p (h c) -> p h c", h=8))
        nc.gpsimd.tensor_copy(
            tv[:, :, 64:65], ones_f[:, None, :].broadcast_to([P, H, 1]))
        vt.append(t)

    # PE order: k-chunks lead their pair's QK by one so the copy of kt[p]
    # is never on the critical path; two v-chunks slot in early to keep
    # the exp chain fed, the remaining four run AFTER the last QK so the
    # exp+mask pipeline fully drains (on ACT/DVE) before the AV phase --
    # otherwise the in-order DVE queue serializes mask(7) ahead of the
    # recip/norm chain and stalls the PE.
    emit_kchunk(0)
    for n in range(1, KC):
        emit_kchunk(n)
        emit_qk(n - 1)
        if n >= 2:
            emit_vchunk(n - 2)
    emit_qk(KC - 1)

    wo = []
    for k in range(KC):
        t = w_pool.tile([P, 1024], F16, tag="w", bufs=16)
        nc.sync.dma_start(t[:], wout_d.ap()[k * P:(k + 1) * P, :])
        wo.append(t)

    # ---- attention values + denominators + normalize ---------------------
    # avout per head: [q, (qb, 65)] with queries on partitions; col 64 of
    # each 65-group is the softmax denominator (from the vt ones column).
    # at layout [q, (pair, qb, sub, d)]: the PE transpose needs each
    # (pair, qb) feature block contiguous (matmul weights APs must have a
    # single free dimension).
    at = at_pool.tile([P, 4096], F16)
    units_by_qb = [[u for u in UNITS if u[1] == qb] for qb in range(QB)]
    atT = []

    def emit_transpose(p):
        psT = sc_pool.tile([P, SQ], F16, tag="sc", name=f"tr{p}")
        for qb in range(QB):
            nc.tensor.transpose(psT[:, qb * P:(qb + 1) * P],
                                at[:, p * 512 + qb * P:p * 512 + (qb + 1) * P],
                                ident[:])
        t = atT_pool.tile([P, SQ], F16, tag="atT")
        if p % 2 == 0:
            nc.scalar.copy(t[:], psT[:])
        else:
            nc.vector.tensor_copy(t[:], psT[:])
        atT.append(t)

    for p in range(KC):
        ev = et[p][:].rearrange("p (s c) -> p s c", s=2)
        avs = []
        for sub in range(2):
            h = 2 * p + sub
            av = sc_pool.tile([P, QB * 65], F32, tag="sc", name=f"av{h}")
            avs.append(av)
            for qb in range(QB):
                us = units_by_qb[qb]
                for i, (T, _, c0) in enumerate(us):
                    nc.tensor.matmul(
                        av[:, qb * 65:qb * 65 + 65],
                        ev[:, sub, c0:c0 + 128],
                        vt[T][:, h * 65:(h + 1) * 65],
                        start=(i == 0), stop=(i == len(us) - 1))
        rc = rc_pool.tile([P, 8], F32, tag="rc")
        at_v = at[:].rearrange("p (pp q s d) -> p pp q s d", pp=KC, q=QB, s=2)
        for sub in range(2):
            nc.vector.reciprocal(
                rc[:, sub * 4:(sub + 1) * 4],
                avs[sub][:].rearrange("p (q c) -> p q c", c=65)[:, :, 64])
        for sub in range(2):
            nc.vector.tensor_tensor(
                at_v[:, p, :, sub, :],
                avs[sub][:].rearrange("p (q c) -> p q c", c=65)[:, :, 0:D],
                rc[:, sub * 4:(sub + 1) * 4][:, :, None]
                    .broadcast_to([P, QB, D]),
                mybir.AluOpType.mult)
    # transpose at [q, f] -> atT [f, q] via PE identity matmuls, batched
    # after the AV loop: the norm chain on DVE drains while the PE runs
    # the remaining AVs, so the transposes rarely wait.
    for p in range(KC):
        emit_transpose(p)

    # ---- output projection ----------------------------------------------
    # Blocks 0..2: one copy per half (ACT||DVE) + one DMA per half (the
    # DMAs overlap later matmuls).  Last block: 256-col column groups so
    # the copy+DMA of each group hides under the next group's matmuls and
    # the serial tail is just the final 256 columns.
    for sb in range(QB - 1):
        psa = sc_pool.tile([P, 512], F32, tag="sc", name=f"opa{sb}")
        psb = sc_pool.tile([P, 512], F32, tag="sc", name=f"opb{sb}")
        for c in range(KC):
            nc.tensor.matmul(psa[:], atT[c][:, sb * P:(sb + 1) * P],
                             wo[c][:, 0:512], start=(c == 0), stop=(c == KC - 1))
            nc.tensor.matmul(psb[:], atT[c][:, sb * P:(sb + 1) * P],
                             wo[c][:, 512:1024], start=(c == 0), stop=(c == KC - 1))
        ob = os_pool.tile([P, E], F16, tag="os")
        nc.scalar.copy(ob[:, 0:512], psa[:])
        nc.vector.tensor_copy(ob[:, 512:1024], psb[:])
        nc.sync.dma_start(out_d.ap()[sb * P:(sb + 1) * P, 0:512],
                          ob[:, 0:512])
        nc.sync.dma_start(out_d.ap()[sb * P:(sb + 1) * P, 512:1024],
                          ob[:, 512:1024])
    # Last block: cols 0:768 first, then 256 cols alone so the final
    # serial copy+DMA tail is only a quarter block.
    sb = QB - 1
    psa = sc_pool.tile([P, 512], F32, tag="sc", name="opa3")
    psb = sc_pool.tile([P, 512], F32, tag="sc", name="opb3")
    ob = os_pool.tile([P, E], F16, tag="os")
    for c in range(KC):
        nc.tensor.matmul(psa[:], atT[c][:, sb * P:(sb + 1) * P],
                         wo[c][:, 0:512], start=(c == 0), stop=(c == KC - 1))
        nc.tensor.matmul(psb[:, 0:256], atT[c][:, sb * P:(sb + 1) * P],
                         wo[c][:, 512:768], start=(c == 0), stop=(c == KC - 1))
    nc.scalar.copy(ob[:, 0:512], psa[:])
    nc.vector.tensor_copy(ob[:, 512:768], psb[:, 0:256])
    nc.sync.dma_start(out_d.ap()[sb * P:(sb + 1) * P, 0:512], ob[:, 0:512])
    nc.sync.dma_start(out_d.ap()[sb * P:(sb + 1) * P, 512:768],
                      ob[:, 512:768])
    for c in range(KC):
        nc.tensor.matmul(psb[:, 256:512], atT[c][:, sb * P:(sb + 1) * P],
                         wo[c][:, 768:1024], start=(c == 0), stop=(c == KC - 1))
    nc.vector.tensor_copy(ob[:, 768:1024], psb[:, 256:512])
    nc.sync.dma_start(out_d.ap()[sb * P:(sb + 1) * P, 768:1024],
                      ob[:, 768:1024])


def build(n_iters: int = 1):
    nc = bacc.Bacc("TRN2", target_bir_lowering=False, debug=False,
                   num_devices=N_CORES)
    xT_d = nc.dram_tensor("xT", [E, SK], F16, kind="ExternalInput")
    wq_d = nc.dram_tensor("wq", [E, E], F16, kind="ExternalInput")
    wk_d = nc.dram_tensor("wk", [E, E], F16, kind="ExternalInput")
    wv_d = nc.dram_tensor("wv", [E, E], F16, kind="ExternalInput")
    wout_d = nc.dram_tensor("wout", [E, E], F16, kind="ExternalInput")
    pmask_d = nc.dram_tensor("pmask", [128, NSLICE_COLS], F16,
                             kind="ExternalInput")
    ident_d = nc.dram_tensor("ident", [128, 128], F16, kind="ExternalInput")
    out_d = nc.dram_tensor("out", [SQ, E], F16, kind="ExternalOutput")
    with tile.TileContext(nc) as tc_, ExitStack() as ctx:
        if n_iters > 1:
            with tc_.For_i(0, n_iters, 1):
                _emit_body(ctx, tc_, xT_d, wq_d, wk_d, wv_d, wout_d, pmask_d,
                           ident_d, out_d, warm=False)
        else:
            _emit_body(ctx, tc_, xT_d, wq_d, wk_d, wv_d, wout_d, pmask_d,
                       ident_d, out_d, warm=True)
    nc.compile()
    return nc


def make_in_maps(x, allowed_mask, w_qkv, w_out):
    """Shard the full inputs into per-core input maps (host marshaling)."""
    x2 = np.asarray(x, dtype=np.float32).reshape(S, E)
    wqkv = np.asarray(w_qkv, dtype=np.float32)
    wq = np.ascontiguousarray(wqkv[:, 0:E]) * np.float32(SCALE)
    wk = np.ascontiguousarray(wqkv[:, E:2 * E])
    wv = np.ascontiguousarray(wqkv[:, 2 * E:3 * E])
    wout = np.ascontiguousarray(np.asarray(w_out, dtype=np.float32))
    am = np.asarray(allowed_mask).reshape(S, S)
    ident = np.eye(128, dtype=np.float16)

    xT = np.ascontiguousarray(x2.T)  # [E, S]
    in_maps = []
    for c in range(N_CORES):
        lo = c * SQ - HALO
        xTc = np.zeros((E, SK), dtype=np.float32)
        ofs = max(0, -lo)
        xTc[:, ofs:] = xT[:, lo + ofs:c * SQ + SQ]
        pmask = np.zeros((128, NSLICE_COLS), dtype=np.float32)
        for (T, qb, col) in UNITS:
            t0 = lo + T * 128
            if t0 + 128 <= 0:
                continue
            tlo = max(0, -t0)
            s0 = c * SQ + qb * 128
            blk = am[s0:s0 + 128, t0 + tlo:t0 + 128]  # [s, t]
            pmask[tlo:128, col:col + 128] = blk.T.astype(np.float32)
        in_maps.append({
            "xT": xTc.astype(np.float16),
            "wq": wq.astype(np.float16),
            "wk": wk.astype(np.float16),
            "wv": wv.astype(np.float16),
            "wout": wout.astype(np.float16),
            "pmask": pmask.astype(np.float16),
            "ident": ident,
        })
    return in_maps


_CACHED_NC = None


def kernel(x, allowed_mask, w_qkv, w_out):
    global _CACHED_NC
    if _CACHED_NC is None:
        _CACHED_NC = build()
    in_maps = make_in_maps(x, allowed_mask, w_qkv, w_out)
    res = run_bass_kernel_spmd(_CACHED_NC, in_maps, list(range(N_CORES)))
    out = np.concatenate([res.results[c]["out"].astype(np.float32)
                          for c in range(N_CORES)], axis=0)
    return out.reshape(B, S, E)
